# revision 75
# baseline (speedup 1.0000x reference)
"""Trainium2 Bass kernel for MultiLatentAttention (MLA).

Sharding: 8 cores = 2 (batch) x 4 (head-groups of 4 heads).
Within each batch group of 4 cores, the down-projections are sharded by
output rows and AllGathered (per S-panel, pipelined).  Each core computes
the shared k_rope head locally for all panels (no collective), runs its
4 heads' up-projections + SDPA and a partial output projection
y_part = attn_out @ Wo[:, heads].T.  Host sums the 4 partials per batch.

Speed tricks vs the bf16 baseline:
- Scores are computed with fp8e4 DoubleRow matmuls: the 192-dim qk
  contraction (128 nope + 64 rope) is packed as [128, 2, *] operands and
  runs at 0.5 cycles/column (4x fewer PE cycles than 2 bf16 matmuls).
- The q up-projection also runs in fp8 DoubleRow (k-tile pairs), with
  the fp8 weights pre-scaled by 64 to stay in e4m3 normal range; the 64
  is divided back out in the psum->q cast on the Act engine.
- c_q travels through the AllGather in fp8 (mixed fp8/bf16 gather buffer
  via bitcast views), shrinking collective bytes and killing the
  readback cast.
- Softmax denominators use DVE pair-sums of e-tiles, halving the
  ones-matmul count; attention output stays in SBUF (no DRAM roundtrip)
  and the output projection is software-pipelined into the SDPA stream.
- 1/sqrt(192) is split as s2 = 192**-0.25 folded into both the q-side
  and k-side weights so fp8 operands are magnitude-balanced.
All fp8/bf16 matmuls accumulate in f32 PSUM; y partials are f32.
"""

import sys

if "/opt/trn_rl_repo" not in sys.path:
    sys.path.insert(0, "/opt/trn_rl_repo")

import numpy as np
import ml_dtypes

BF16 = ml_dtypes.bfloat16
F8 = ml_dtypes.float8_e4m3

B, S, D, H = 2, 2048, 2048, 16
QR, KVR = 1536, 512
NOPE, RD, VD = 128, 64, 128
QK_D = NOPE + RD
HL = 4          # heads per core
G = 4           # head groups (= cores per batch group)
QSH = QR // G   # 384 c_q rows per core
KSH = KVR // G  # 128 c_kv rows per core
PAN = 512       # panel width
P = 128
GR = 320        # gather rows (bf16 units): 192 (cq fp8) + 128 (ckv bf16)

_cache = {}


def _build_module(phases="ABCD"):
    import concourse.bacc as bacc
    import concourse.mybir as mybir
    import concourse.tile as tile

    dt = mybir.dt
    f32, bf16, f8 = dt.float32, dt.bfloat16, dt.float8e4
    AF = mybir.ActivationFunctionType
    DR = mybir.MatmulPerfMode.DoubleRow

    nc = bacc.Bacc("TRN2", target_bir_lowering=False, debug=False, num_devices=8)

    def inp(name, shape, dtype=bf16):
        return nc.dram_tensor(name, shape, dtype, kind="ExternalInput").ap()

    # all inputs arrive pre-arranged partition-major ([128, flat]) so every
    # load is a contiguous 2-D DMA (128 descriptors, no strided gather)
    xp = inp("xp", [P, 4 * 16 * PAN])       # x[b].T as [p, panel, kt, s]
    xp8 = inp("xp8", [P, 4 * 16 * PAN], f8)
    wqd8 = inp("wqd8", [P, 8 * 2 * QSH], f8)   # Wq_down.T slice * 64, [p,j,two,m]
    wkvd = inp("wkvd", [P, 16 * KSH])       # Wkv_down.T slice, [p,kt,m]
    wkr = inp("wkr", [P, 16 * RD])          # Wk_rope.T * s2, [p,kt,m]
    wqf = inp("wqf", [P, 6 * 2 * 768], f8)  # [Wq_up|Wq_rope].T * s2*64
    wku = inp("wku", [P, 4 * 512])          # Wk_up_g.T * s2, [p,kt,m]
    wvu = inp("wvu", [P, 4 * 512])          # Wv_up_g.T
    wo = inp("wo", [P, 4 * D])              # Wo[:, cols_g].T, [p,kt,m]
    cosb = inp("cosb", [P, S])              # bf16 rope tables, 4x partition-tiled
    sinb = inp("sinb", [P, S])
    masks = inp("masks", [P, G * PAN])      # multiplicative causal masks
    onc = inp("onc", [P, 1])                # ones column
    y = nc.dram_tensor("y", [S, D], bf16, kind="ExternalOutput").ap()

    KT_D = D // P      # 16 k-tiles over model dim
    KT_KV = KVR // P   # 4
    NP = S // PAN      # 4 panels
    GROUPS = [[0, 1, 2, 3], [4, 5, 6, 7]]
    INV64 = 1.0 / 64.0

    with tile.TileContext(nc) as tc:
        with (
            tc.tile_pool(name="res", bufs=1) as res,
            tc.tile_pool(name="panels", bufs=8) as panels,
            tc.tile_pool(name="work", bufs=2) as work,
            tc.tile_pool(name="dram", bufs=1, space="DRAM") as dram,
        ):
            # ---- SBUF residents ----------------------------------------
            # q/k fp8 layout [P, head, slot, S]: slot 0 = nope dims,
            # slot 1 rows 0:64 = roped rope dims, rows 64:128 zero.
            q_sb = res.tile([P, HL, 2, S], f8, tag="q")
            k_sb = res.tile([P, HL, 2, S], f8, tag="k")
            v_sb = res.tile([P, S // P, 512], bf16, tag="v")
            ao_sb = res.tile([P, HL, S], bf16, tag="ao")
            masks_sb = res.tile([P, G, PAN], bf16, tag="masks")
            onc_sb = res.tile([P, 1], bf16, tag="onc")
            cos_sb = res.tile([P, S], bf16, tag="cos")
            sin_sb = res.tile([P, S], bf16, tag="sin")

            # zero the unused upper rope rows so the fp8 matmul never sees
            # garbage (e4m3 NaN patterns would poison 0*NaN)
            nc.gpsimd.memset(q_sb[64:128, :, 1, :], 0.0)
            nc.gpsimd.memset(k_sb[64:128, :, 1, :], 0.0)
            nc.gpsimd.dma_start(
                masks_sb[:], masks.rearrange("p (j q) -> p j q", q=PAN)
            )
            nc.gpsimd.dma_start(onc_sb[:], onc[:])
            nc.gpsimd.dma_start(cos_sb[:], cosb[:])
            nc.gpsimd.dma_start(sin_sb[:], sinb[:])

            # ---- DRAM staging for the latent AllGather ------------------
            ag_in = [dram.tile([GR, PAN], bf16, tag=f"agi{n}", name=f"agi{n}")
                     for n in range(NP)]
            ag_out = [dram.tile([G * GR, PAN], bf16, tag=f"ago{n}",
                                name=f"ago{n}") for n in range(NP)]

            def f8rows(t):
                # [rows, PAN] bf16 -> [2*rows, PAN] fp8 view of same bytes
                return t[:].bitcast(f8).rearrange("r (two s) -> (r two) s",
                                                  s=PAN)

            def rope_block(dst64, src64, ns, b0):
                # dst [64, PAN] fp8 slot-1 rows; src [64, PAN] bf16 whose
                # halves sit at absolute partitions b0 and b0+32 (the BIR
                # verifier wants matching base partitions for SBUF inputs,
                # hence the partition-tiled cos/sin tables)
                cs0, cs1 = cos_sb[b0 : b0 + 32, ns], cos_sb[b0 + 32 : b0 + 64, ns]
                sn0, sn1 = sin_sb[b0 : b0 + 32, ns], sin_sb[b0 + 32 : b0 + 64, ns]
                t1 = work.tile([32, PAN], bf16, tag="rt1")
                t2 = work.tile([32, PAN], bf16, tag="rt2")
                nc.vector.tensor_mul(t1, src64[0:32, :], cs0)
                nc.vector.tensor_mul(t2, src64[32:64, :], sn1)
                nc.vector.tensor_sub(dst64[0:32, :], t1, t2)
                t3 = work.tile([32, PAN], bf16, tag="rt1")
                t4 = work.tile([32, PAN], bf16, tag="rt2")
                nc.vector.tensor_mul(t3, src64[32:64, :], cs1)
                nc.vector.tensor_mul(t4, src64[0:32, :], sn0)
                nc.vector.tensor_add(dst64[32:64, :], t3, t4)

            # ---- Phase A + B -------------------------------------------
            with (
                tc.tile_pool(name="pa", bufs=1) as pa,
                tc.tile_pool(name="pb", bufs=1) as pb,
                tc.tile_pool(name="bq", bufs=3) as bq,
                tc.tile_pool(name="psA", bufs=4, space="PSUM") as psA,
                tc.tile_pool(name="psB", bufs=3, space="PSUM") as psB,
            ):
                # A weights: wqd8 chunked so the first matmul starts early
                wqd_sb = pa.tile([P, KT_D // 2, 2, QSH], f8, tag="wqd8")
                x80_ch = []
                for c in range(4):
                    nc.sync.dma_start(wqd_sb[:, 2 * c : 2 * c + 2, :, :],
                                      wqd8[:, c * 2 * 2 * QSH : (c + 1) * 2 * 2 * QSH])
                    t = panels.tile([P, 2, 2, PAN], f8, tag="panel8",
                                    name=f"x8_sb0_{c}")
                    nc.sync.dma_start(t[:], xp8[:, c * 4 * PAN : (c + 1) * 4 * PAN])
                    x80_ch.append(t)
                wkvd_sb = pa.tile([P, KT_D, KSH], bf16, tag="wkvd")
                nc.sync.dma_start(wkvd_sb[:], wkvd[:])
                wkr_sb = pa.tile([P, KT_D, RD], bf16, tag="wkr")
                nc.sync.dma_start(wkr_sb[:], wkr[:])
                x0_ch = []
                for c in range(4):
                    t = panels.tile([P, 4, PAN], bf16, tag="panel",
                                    name=f"x_sb0_{c}")
                    (nc.sync if c % 2 == 0 else nc.scalar).dma_start(
                        t[:], xp[:, c * 4 * PAN : (c + 1) * 4 * PAN])
                    x0_ch.append(t)

                # B weights on the gpsimd DMA queue, off the hot SP queue
                wqf_sb = pb.tile([P, 6, 2, 768], f8, tag="wqf")
                nc.gpsimd.dma_start(wqf_sb[:], wqf[:])
                wku_sb = pb.tile([P, KT_KV, 512], bf16, tag="wku")
                nc.gpsimd.dma_start(wku_sb[:], wku[:])
                wvu_sb = pb.tile([P, KT_KV, 512], bf16, tag="wvu")
                nc.gpsimd.dma_start(wvu_sb[:], wvu[:])

                def phase_a(n):
                    """my latent slices for panel n + local k_rope, then gather"""
                    ns = slice(n * PAN, (n + 1) * PAN)
                    if n == 0:
                        x_ch, x8_ch = x0_ch, x80_ch
                    else:
                        x8_ch = []
                        for c in range(4):
                            t = panels.tile([P, 2, 2, PAN], f8, tag="panel8",
                                            name=f"x8_sb{n}_{c}")
                            o = (n * 16 + c * 4) * PAN
                            nc.sync.dma_start(t[:], xp8[:, o : o + 4 * PAN])
                            x8_ch.append(t)
                        x_ch = []
                        for c in range(4):
                            t = panels.tile([P, 4, PAN], bf16, tag="panel",
                                            name=f"x_sb{n}_{c}")
                            o = (n * 16 + c * 4) * PAN
                            (nc.sync if c % 2 == 0 else nc.scalar).dma_start(
                                t[:], xp[:, o : o + 4 * PAN])
                            x_ch.append(t)
                    gin8 = f8rows(ag_in[n])
                    # c_q in fp8 DoubleRow: kt-pair-major over 3 concurrent
                    # psums so matmuls start as soon as the first chunk lands
                    pss = [psA.tile([P, PAN], f32, tag="psA", name=f"psA{n}_{m}")
                           for m in range(4)]
                    for j in range(KT_D // 2):
                        xk = x8_ch[j // 2][:, j % 2, :, :]
                        st0, st1 = (j == 0), (j == KT_D // 2 - 1)
                        for m in range(3):
                            nc.tensor.matmul(
                                pss[m],
                                lhsT=wqd_sb[:, j, :, m * P : (m + 1) * P],
                                rhs=xk, start=st0, stop=st1, perf_mode=DR,
                            )
                    # c_kv (bf16, feeds v) + local k_rope share the x chunks
                    ps_kr = psA.tile([64, PAN], f32, tag="psKR", bufs=1)
                    for kt in range(KT_D):
                        xk = x_ch[kt // 4][:, kt % 4, :]
                        st0, st1 = (kt == 0), (kt == KT_D - 1)
                        nc.tensor.matmul(pss[3], lhsT=wkvd_sb[:, kt, :],
                                         rhs=xk, start=st0, stop=st1)
                        nc.tensor.matmul(ps_kr, lhsT=wkr_sb[:, kt, :],
                                         rhs=xk, start=st0, stop=st1)
                    cqr = gin8[0:384, :].rearrange("(p r) s -> p r s", r=3)
                    for m in range(3):  # c_q -> fp8 staging (scale 1/64 back)
                        st = work.tile([P, PAN], f8, tag="cq8")
                        nc.scalar.activation(st, pss[m], AF.Copy, scale=INV64)
                        nc.sync.dma_start(cqr[:, m, :], st)
                    st = work.tile([P, PAN], bf16, tag="ckvst")
                    nc.scalar.activation(st, pss[3], AF.Copy)
                    nc.sync.dma_start(ag_in[n][192:320, :], st)
                    nc.gpsimd.collective_compute(
                        "AllGather", mybir.AluOpType.bypass,
                        replica_groups=GROUPS,
                        ins=[ag_in[n].opt()], outs=[ag_out[n].opt()],
                    )
                    krb = work.tile([64, PAN], bf16, tag="krb")
                    nc.scalar.activation(krb, ps_kr, AF.Copy)
                    rope_block(k_sb[0:64, 0, 1, ns], krb, ns, 0)
                    for hh in range(1, HL):  # replicate to the other heads
                        nc.vector.tensor_copy(k_sb[0:64, hh, 1, ns],
                                              k_sb[0:64, 0, 1, ns])

                def phase_b(n):
                    """up-projections for panel n from the gathered latents"""
                    ns = slice(n * PAN, (n + 1) * PAN)
                    gout8 = f8rows(ag_out[n])
                    cqf = bq.tile([P, 12, PAN], f8, tag="cqf")
                    nc.scalar.dma_start(
                        cqf[:],
                        gout8.rearrange("(gi x) s -> gi x s", x=2 * GR)[
                            :, 0:QSH, :
                        ].rearrange("gi (p r) s -> p gi r s", r=3),
                    )
                    ckv = bq.tile([P, KT_KV, PAN], bf16, tag="ckv")
                    nc.scalar.dma_start(
                        ckv[:],
                        ag_out[n].rearrange("(gi x) s -> gi x s", x=GR)[
                            :, 192:320, :
                        ].rearrange("gi p s -> p gi s"),
                    )
                    for m in range(HL):  # q nope heads (fp8 DoubleRow)
                        ps = psB.tile([P, PAN], f32, tag="psB")
                        for j in range(6):
                            nc.tensor.matmul(
                                ps,
                                lhsT=wqf_sb[:, j, :, m * P : (m + 1) * P],
                                rhs=cqf[:, 2 * j : 2 * j + 2, :],
                                start=(j == 0), stop=(j == 5), perf_mode=DR,
                            )
                        nc.scalar.activation(q_sb[:, m, 0, ns], ps, AF.Copy,
                                             scale=INV64)
                    for hp in range(HL // 2):  # rope: two heads per psum
                        c0 = 512 + 128 * hp
                        ps = psB.tile([P, PAN], f32, tag="psB")
                        for j in range(6):
                            nc.tensor.matmul(
                                ps,
                                lhsT=wqf_sb[:, j, :, c0 : c0 + 128],
                                rhs=cqf[:, 2 * j : 2 * j + 2, :],
                                start=(j == 0), stop=(j == 5), perf_mode=DR,
                            )
                        qrp = work.tile([P, PAN], bf16, tag="qrp")
                        nc.scalar.activation(qrp, ps, AF.Copy, scale=INV64)
                        rope_block(q_sb[0:64, 2 * hp, 1, ns], qrp[0:64, :], ns, 0)
                        rope_block(q_sb[0:64, 2 * hp + 1, 1, ns],
                                   qrp[64:128, :], ns, 64)
                    for m in range(HL):  # k_c
                        ps = psB.tile([P, PAN], f32, tag="psB")
                        for kt in range(KT_KV):
                            nc.tensor.matmul(
                                ps,
                                lhsT=wku_sb[:, kt, m * P : (m + 1) * P],
                                rhs=ckv[:, kt, :],
                                start=(kt == 0), stop=(kt == KT_KV - 1),
                            )
                        nc.scalar.activation(k_sb[:, m, 0, ns], ps, AF.Copy)
                    for sti in range(4):  # v for this panel's S-tiles
                        ps = psB.tile([P, PAN], f32, tag="psB")
                        for kt in range(KT_KV):
                            nc.tensor.matmul(
                                ps,
                                lhsT=ckv[:, kt, sti * P : (sti + 1) * P],
                                rhs=wvu_sb[:, kt, :],
                                start=(kt == 0), stop=(kt == KT_KV - 1),
                            )
                        nc.scalar.activation(v_sb[:, 4 * n + sti, :], ps, AF.Copy)

                phase_a(0)
                phase_a(1)
                phase_a(2)
                phase_b(0)
                phase_a(3)
                phase_b(1)
                phase_b(2)
                phase_b(3)

            # ---------------- Phase C: SDPA + Phase D interleaved --------
            if "C" not in phases:
                # timing-partial build: consume B outputs so nothing is elided
                nc.gpsimd.dma_start(y[0:P, 0:PAN], q_sb[:, 0, 0, 0:PAN])
                nc.gpsimd.dma_start(y[P : 2 * P, 0:PAN], k_sb[:, 0, 0, 0:PAN])
                nc.gpsimd.dma_start(y[2 * P : 3 * P, 0:PAN], v_sb[:, 0, 0:PAN])
            elif True:
                do_cd(tc, nc, phases, q_sb, k_sb, v_sb, ao_sb, masks_sb,
                      onc_sb, wo, work, y)

    nc.compile()
    return nc


def do_cd(tc, nc, phases, q_sb, k_sb, v_sb, ao_sb, masks_sb, onc_sb, wo,
          work, y):
    import concourse.mybir as mybir

    dt = mybir.dt
    f32, bf16 = dt.float32, dt.bfloat16
    AF = mybir.ActivationFunctionType
    DR = mybir.MatmulPerfMode.DoubleRow
    if True:
        if True:
            with (
                tc.tile_pool(name="pw", bufs=1) as pw,
                tc.tile_pool(name="pe", bufs=8) as pe,
                tc.tile_pool(name="pds", bufs=2) as pds,
                tc.tile_pool(name="py", bufs=4) as py,
                tc.tile_pool(name="psS", bufs=2, space="PSUM") as psS,
                tc.tile_pool(name="psO", bufs=2, space="PSUM") as psO,
                tc.tile_pool(name="psDn", bufs=1, space="PSUM") as psDn,
                tc.tile_pool(name="psD", bufs=3, space="PSUM") as psD,
            ):
                wo_sb = pw.tile([P, HL, D], bf16, tag="wo")
                nc.gpsimd.dma_start(wo_sb[:], wo[:])
                ycnt = [0]

                def d_unit(m, nh, tail=False):
                    """one output-projection unit: 2 nn-blocks of row-tile m"""
                    ms = slice(m * P, (m + 1) * P)
                    yst = py.tile([P, 2 * PAN], bf16, tag="yst")
                    for sub in range(2):
                        nn = 2 * nh + sub
                        ps = psD.tile([P, PAN], f32, tag="psD")
                        for kt in range(HL):
                            nc.tensor.matmul(
                                ps,
                                lhsT=ao_sb[:, kt, ms],
                                rhs=wo_sb[:, kt, nn * PAN : (nn + 1) * PAN],
                                start=(kt == 0), stop=(kt == HL - 1),
                            )
                        nc.vector.tensor_copy(
                            yst[:, sub * PAN : (sub + 1) * PAN], ps)
                    q = (nc.sync, nc.gpsimd, nc.scalar)[ycnt[0] % 3]
                    ycnt[0] += 1
                    q.dma_start(y[ms, 2 * nh * PAN : (2 * nh + 2) * PAN], yst)

                dq = []  # pending D units, drained inside the SDPA stream

                def d_fill():
                    if dq and "D" in phases:
                        m, nh = dq.pop(0)
                        d_unit(m, nh)

                def sdpa_pair(g, h0):
                    """two heads' SDPA chains interleaved kb-by-kb so the
                    in-order PE queue always has independent ready work while
                    the other chain waits on its exp"""
                    gs = slice(g * PAN, (g + 1) * PAN)
                    nk = 4 * (g + 1)
                    hs = (h0, h0 + 1)
                    ps_o = {h: psO.tile([P, PAN], f32, tag="ps_o",
                                        name=f"pso{g}_{h}") for h in hs}
                    # both chains' denominators live in one psum bank, at
                    # partition bases 0 and 32 (legal matmul tile positions)
                    ps_d2 = psDn.tile([33, PAN], f32, tag="ps_d")
                    e_t = {h: [None] * nk for h in hs}
                    dsum = {h: [None] * (nk // 2) for h in hs}
                    qsum = {h: [None] * (nk // 4) for h in hs}
                    pend = {h: [] for h in hs}

                    def consume(ci, h, kb):
                        nc.tensor.matmul(
                            ps_o[h], lhsT=v_sb[:, kb, h * P : (h + 1) * P],
                            rhs=e_t[h][kb],
                            start=(kb == 0), stop=(kb == nk - 1),
                        )
                        if kb % 4 == 3:
                            nc.tensor.matmul(
                                ps_d2[32 * ci : 32 * ci + 1, :],
                                lhsT=onc_sb[:], rhs=qsum[h][kb // 4],
                                start=(kb == 3), stop=(kb == nk - 1),
                                skip_group_check=True,
                            )

                    for kb in range(nk):
                        ks = slice(kb * P, (kb + 1) * P)
                        for ci, h in enumerate(hs):
                            ps_s = psS.tile([P, PAN], f32, tag="ps_s")
                            nc.tensor.matmul(
                                ps_s, lhsT=k_sb[:, h, :, ks],
                                rhs=q_sb[:, h, :, gs],
                                start=True, stop=True, perf_mode=DR,
                            )
                            e = pe.tile([P, PAN], bf16, tag="e")
                            nc.scalar.activation(e, ps_s, AF.Exp)
                            if kb >= 4 * g:
                                nc.vector.tensor_mul(
                                    e, e, masks_sb[:, kb - 4 * g, :])
                            e_t[h][kb] = e
                            if kb % 2 == 1:
                                ds = pds.tile([P, PAN], bf16, tag=f"dsum{ci}")
                                nc.vector.tensor_add(ds, e_t[h][kb - 1],
                                                     e_t[h][kb])
                                dsum[h][kb // 2] = ds
                            if kb % 4 == 3:
                                qs = pds.tile([P, PAN], bf16, tag=f"qsum{ci}")
                                nc.vector.tensor_add(qs, dsum[h][kb // 2 - 1],
                                                     dsum[h][kb // 2])
                                qsum[h][kb // 4] = qs
                            pend[h].append(kb)
                            if len(pend[h]) > 2:
                                consume(ci, h, pend[h].pop(0))
                    for ci, h in enumerate(hs):
                        while pend[h]:
                            consume(ci, h, pend[h].pop(0))
                    for ci, h in enumerate(hs):
                        rc = work.tile([1, PAN], f32, tag="rc")
                        nc.vector.reciprocal(rc, ps_d2[32 * ci : 32 * ci + 1, :])
                        bb = work.tile([P, PAN], f32, tag="bb")
                        nc.gpsimd.partition_broadcast(bb, rc)
                        nc.vector.tensor_mul(ao_sb[:, h, gs], ps_o[h], bb)

                for g in range(G):
                    if g >= 1:
                        dq.extend((4 * (g - 1) + mm, nh)
                                  for mm in range(4) for nh in range(2))
                    for hp in range(HL // 2):
                        sdpa_pair(g, 2 * hp)
                        for _ in range(4):
                            d_fill()
                    while dq:
                        d_fill()
                if "D" in phases:
                    for m in range(4 * (G - 1), 4 * G):
                        for nh in range(2):
                            d_unit(m, nh, tail=True)


def _prep_inputs(x, positions, Wq_down, Wq_up, Wq_rope, Wkv_down, Wk_up, Wv_up,
                 Wk_rope, Wo):
    s2 = np.float32(QK_D ** -0.25)  # sqrt of the 1/sqrt(qk_d) scale
    bf = lambda a: np.ascontiguousarray(a).astype(BF16)

    def pmaj(a, kt):
        # [kt*128, m] -> [128, kt*m] partition-major
        m = a.shape[1]
        return np.ascontiguousarray(
            a.reshape(kt, P, m).transpose(1, 0, 2).reshape(P, kt * m))

    inv_freq = 1.0 / (10000.0 ** (np.arange(0, RD, 2, dtype=np.float32) / RD))
    ang = positions.astype(np.float32)[:, None] * inv_freq  # (S, 32)
    shared = {
        "wkr": pmaj(Wk_rope.T * s2, 16).astype(BF16),
        "onc": np.ones((P, 1), BF16),
        "cosb": bf(np.tile(np.cos(ang).T, (4, 1))),
        "sinb": bf(np.tile(np.sin(ang).T, (4, 1))),
    }

    mk = np.zeros((P, G * PAN), np.float32)
    for j in range(G):
        p = np.arange(P)[:, None]
        q = np.arange(PAN)[None, :]
        mk[:, j * PAN : (j + 1) * PAN] = (j * P + p <= q).astype(np.float32)
    shared["masks"] = mk.astype(BF16)

    wqdT = Wq_down.T  # (D, QR)
    wkvdT = Wkv_down.T  # (D, KVR)
    per_g = []
    for g in range(G):
        rs, rr = slice(512 * g, 512 * (g + 1)), slice(256 * g, 256 * (g + 1))
        wqall = np.concatenate([Wq_up[rs].T, Wq_rope[rr].T], axis=1)
        per_g.append({
            "wqd8": pmaj(wqdT[:, QSH * g : QSH * (g + 1)] * 64.0, 16).astype(F8),
            "wkvd": pmaj(wkvdT[:, KSH * g : KSH * (g + 1)], 16).astype(BF16),
            "wqf": pmaj(wqall * (s2 * 64.0), 12).astype(F8),
            "wku": pmaj(Wk_up[rs].T * s2, 4).astype(BF16),
            "wvu": pmaj(Wv_up[rs].T, 4).astype(BF16),
            "wo": pmaj(Wo[:, rs].T, 4).astype(BF16),
        })
    # x as [p, panel, kt, s] partition-major flat
    def xprep(a):
        # a: [D, S] -> [128, NP*16*PAN]
        return np.ascontiguousarray(
            a.reshape(16, P, 4, PAN).transpose(1, 2, 0, 3).reshape(P, -1))

    xTs = [x[b].T for b in range(B)]
    xps = [xprep(a).astype(BF16) for a in xTs]
    xp8s = [xprep(a).astype(F8) for a in xTs]

    in_maps = []
    for c in range(8):
        b, g = c // G, c % G
        m = dict(shared)
        m.update(per_g[g])
        m["xp"] = xps[b]
        m["xp8"] = xp8s[b]
        in_maps.append(m)
    return in_maps


def kernel(**inputs):
    from concourse.bass_utils import run_bass_kernel_spmd

    if "nc" not in _cache:
        _cache["nc"] = _build_module()
    nc = _cache["nc"]

    in_maps = _prep_inputs(**inputs)
    res = None
    for attempt in range(3):
        try:
            res = run_bass_kernel_spmd(nc, in_maps, core_ids=list(range(8)))
            break
        except Exception:
            if attempt == 2:
                raise
    out = np.zeros((B, S, D), np.float32)
    for c in range(8):
        out[c // G] += res.results[c]["y"].astype(np.float32)
    return out


# revision 80
# speedup vs baseline: 1.0032x; 1.0032x over previous
"""Trainium2 Bass kernel for MultiLatentAttention (MLA).

Sharding: 8 cores = 2 (batch) x 4 (head-groups of 4 heads).
Within each batch group of 4 cores, the down-projections are sharded by
output rows and AllGathered (per S-panel, pipelined).  Each core computes
the shared k_rope head locally for all panels (no collective), runs its
4 heads' up-projections + SDPA and a partial output projection
y_part = attn_out @ Wo[:, heads].T.  Host sums the 4 partials per batch.

Speed tricks vs the bf16 baseline:
- Scores are computed with fp8e4 DoubleRow matmuls: the 192-dim qk
  contraction (128 nope + 64 rope) is packed as [128, 2, *] operands and
  runs at 0.5 cycles/column (4x fewer PE cycles than 2 bf16 matmuls).
- The q up-projection also runs in fp8 DoubleRow (k-tile pairs), with
  the fp8 weights pre-scaled by 64 to stay in e4m3 normal range; the 64
  is divided back out in the psum->q cast on the Act engine.
- c_q travels through the AllGather in fp8 (mixed fp8/bf16 gather buffer
  via bitcast views), shrinking collective bytes and killing the
  readback cast.
- Softmax denominators use DVE pair-sums of e-tiles, halving the
  ones-matmul count; attention output stays in SBUF (no DRAM roundtrip)
  and the output projection is software-pipelined into the SDPA stream.
- 1/sqrt(192) is split as s2 = 192**-0.25 folded into both the q-side
  and k-side weights so fp8 operands are magnitude-balanced.
All fp8/bf16 matmuls accumulate in f32 PSUM; y partials are f32.
"""

import sys

if "/opt/trn_rl_repo" not in sys.path:
    sys.path.insert(0, "/opt/trn_rl_repo")

import numpy as np
import ml_dtypes

BF16 = ml_dtypes.bfloat16
F8 = ml_dtypes.float8_e4m3

B, S, D, H = 2, 2048, 2048, 16
QR, KVR = 1536, 512
NOPE, RD, VD = 128, 64, 128
QK_D = NOPE + RD
HL = 4          # heads per core
G = 4           # head groups (= cores per batch group)
QSH = QR // G   # 384 c_q rows per core
KSH = KVR // G  # 128 c_kv rows per core
PAN = 512       # panel width
P = 128
GR = 320        # gather rows (bf16 units): 192 (cq fp8) + 128 (ckv bf16)

_cache = {}


def _build_module(phases="ABCD"):
    import concourse.bacc as bacc
    import concourse.mybir as mybir
    import concourse.tile as tile

    dt = mybir.dt
    f32, bf16, f8 = dt.float32, dt.bfloat16, dt.float8e4
    AF = mybir.ActivationFunctionType
    DR = mybir.MatmulPerfMode.DoubleRow

    nc = bacc.Bacc("TRN2", target_bir_lowering=False, debug=False, num_devices=8)

    def inp(name, shape, dtype=bf16):
        return nc.dram_tensor(name, shape, dtype, kind="ExternalInput").ap()

    # all inputs arrive pre-arranged partition-major ([128, flat]) so every
    # load is a contiguous 2-D DMA (128 descriptors, no strided gather)
    xp = inp("xp", [P, 4 * 16 * PAN])       # x[b].T as [p, panel, kt, s]
    xp8 = inp("xp8", [P, 4 * 16 * PAN], f8)
    wqd8 = inp("wqd8", [P, 8 * 2 * QSH], f8)   # Wq_down.T slice * 64, [p,j,two,m]
    wkvd = inp("wkvd", [P, 16 * KSH])       # Wkv_down.T slice, [p,kt,m]
    wkr = inp("wkr", [P, 16 * RD])          # Wk_rope.T * s2, [p,kt,m]
    wqf = inp("wqf", [P, 6 * 2 * 768], f8)  # [Wq_up|Wq_rope].T * s2*64
    wku = inp("wku", [P, 4 * 512])          # Wk_up_g.T * s2, [p,kt,m]
    wvu = inp("wvu", [P, 4 * 512])          # Wv_up_g.T
    wo = inp("wo", [P, 4 * D])              # Wo[:, cols_g].T, [p,kt,m]
    cosb = inp("cosb", [P, S])              # bf16 rope tables, 4x partition-tiled
    sinb = inp("sinb", [P, S])
    masks = inp("masks", [P, G * PAN])      # multiplicative causal masks
    onc = inp("onc", [P, 1])                # ones column
    y = nc.dram_tensor("y", [S, D], bf16, kind="ExternalOutput").ap()

    KT_D = D // P      # 16 k-tiles over model dim
    KT_KV = KVR // P   # 4
    NP = S // PAN      # 4 panels
    GROUPS = [[0, 1, 2, 3], [4, 5, 6, 7]]
    INV64 = 1.0 / 64.0

    with tile.TileContext(nc) as tc:
        with (
            tc.tile_pool(name="res", bufs=1) as res,
            tc.tile_pool(name="panels", bufs=8) as panels,
            tc.tile_pool(name="work", bufs=2) as work,
            tc.tile_pool(name="dram", bufs=1, space="DRAM") as dram,
        ):
            # ---- SBUF residents ----------------------------------------
            # q/k fp8 layout [P, head, slot, S]: slot 0 = nope dims,
            # slot 1 rows 0:64 = roped rope dims, rows 64:128 zero.
            q_sb = res.tile([P, HL, 2, S], f8, tag="q")
            k_sb = res.tile([P, HL, 2, S], f8, tag="k")
            v_sb = res.tile([P, S // P, 512], bf16, tag="v")
            ao_sb = res.tile([P, HL, S], bf16, tag="ao")
            masks_sb = res.tile([P, G, PAN], bf16, tag="masks")
            onc_sb = res.tile([P, 1], bf16, tag="onc")
            cos_sb = res.tile([P, S], bf16, tag="cos")
            sin_sb = res.tile([P, S], bf16, tag="sin")

            # zero the unused upper rope rows so the fp8 matmul never sees
            # garbage (e4m3 NaN patterns would poison 0*NaN)
            nc.gpsimd.memset(q_sb[64:128, :, 1, :], 0.0)
            nc.gpsimd.memset(k_sb[64:128, :, 1, :], 0.0)
            nc.gpsimd.dma_start(
                masks_sb[:], masks.rearrange("p (j q) -> p j q", q=PAN)
            )
            nc.gpsimd.dma_start(onc_sb[:], onc[:])
            nc.gpsimd.dma_start(cos_sb[:], cosb[:])
            nc.gpsimd.dma_start(sin_sb[:], sinb[:])

            # ---- DRAM staging for the latent AllGather ------------------
            ag_in = [dram.tile([GR, PAN], bf16, tag=f"agi{n}", name=f"agi{n}")
                     for n in range(NP)]
            ag_out = [dram.tile([G * GR, PAN], bf16, tag=f"ago{n}",
                                name=f"ago{n}") for n in range(NP)]

            def f8rows(t):
                # [rows, PAN] bf16 -> [2*rows, PAN] fp8 view of same bytes
                return t[:].bitcast(f8).rearrange("r (two s) -> (r two) s",
                                                  s=PAN)

            def rope_block(dst64, src64, ns, b0):
                # dst [64, PAN] fp8 slot-1 rows; src [64, PAN] bf16 whose
                # halves sit at absolute partitions b0 and b0+32 (the BIR
                # verifier wants matching base partitions for SBUF inputs,
                # hence the partition-tiled cos/sin tables)
                cs0, cs1 = cos_sb[b0 : b0 + 32, ns], cos_sb[b0 + 32 : b0 + 64, ns]
                sn0, sn1 = sin_sb[b0 : b0 + 32, ns], sin_sb[b0 + 32 : b0 + 64, ns]
                t1 = work.tile([32, PAN], bf16, tag="rt1")
                t2 = work.tile([32, PAN], bf16, tag="rt2")
                nc.vector.tensor_mul(t1, src64[0:32, :], cs0)
                nc.vector.tensor_mul(t2, src64[32:64, :], sn1)
                nc.vector.tensor_sub(dst64[0:32, :], t1, t2)
                t3 = work.tile([32, PAN], bf16, tag="rt1")
                t4 = work.tile([32, PAN], bf16, tag="rt2")
                nc.vector.tensor_mul(t3, src64[32:64, :], cs1)
                nc.vector.tensor_mul(t4, src64[0:32, :], sn0)
                nc.vector.tensor_add(dst64[32:64, :], t3, t4)

            # ---- Phase A + B -------------------------------------------
            with (
                tc.tile_pool(name="pa", bufs=1) as pa,
                tc.tile_pool(name="pb", bufs=1) as pb,
                tc.tile_pool(name="bq", bufs=3) as bq,
                tc.tile_pool(name="psA", bufs=4, space="PSUM") as psA,
                tc.tile_pool(name="psB", bufs=3, space="PSUM") as psB,
            ):
                # A weights: wqd8 chunked so the first matmul starts early
                wqd_sb = pa.tile([P, KT_D // 2, 2, QSH], f8, tag="wqd8")
                x80_ch = []
                for c in range(4):
                    nc.sync.dma_start(wqd_sb[:, 2 * c : 2 * c + 2, :, :],
                                      wqd8[:, c * 2 * 2 * QSH : (c + 1) * 2 * 2 * QSH])
                    t = panels.tile([P, 2, 2, PAN], f8, tag="panel8",
                                    name=f"x8_sb0_{c}")
                    nc.sync.dma_start(t[:], xp8[:, c * 4 * PAN : (c + 1) * 4 * PAN])
                    x80_ch.append(t)
                wkvd_sb = pa.tile([P, KT_D, KSH], bf16, tag="wkvd")
                nc.sync.dma_start(wkvd_sb[:], wkvd[:])
                wkr_sb = pa.tile([P, KT_D, RD], bf16, tag="wkr")
                nc.sync.dma_start(wkr_sb[:], wkr[:])
                x0_ch = []
                for c in range(4):
                    t = panels.tile([P, 4, PAN], bf16, tag="panel",
                                    name=f"x_sb0_{c}")
                    (nc.sync if c % 2 == 0 else nc.scalar).dma_start(
                        t[:], xp[:, c * 4 * PAN : (c + 1) * 4 * PAN])
                    x0_ch.append(t)

                # B weights on the gpsimd DMA queue, off the hot SP queue
                wqf_sb = pb.tile([P, 6, 2, 768], f8, tag="wqf")
                nc.gpsimd.dma_start(wqf_sb[:], wqf[:])
                wku_sb = pb.tile([P, KT_KV, 512], bf16, tag="wku")
                nc.gpsimd.dma_start(wku_sb[:], wku[:])
                wvu_sb = pb.tile([P, KT_KV, 512], bf16, tag="wvu")
                nc.gpsimd.dma_start(wvu_sb[:], wvu[:])

                def phase_a(n):
                    """my latent slices for panel n + local k_rope, then gather"""
                    ns = slice(n * PAN, (n + 1) * PAN)
                    if n == 0:
                        x_ch, x8_ch = x0_ch, x80_ch
                    else:
                        x8_ch = []
                        for c in range(4):
                            t = panels.tile([P, 2, 2, PAN], f8, tag="panel8",
                                            name=f"x8_sb{n}_{c}")
                            o = (n * 16 + c * 4) * PAN
                            nc.sync.dma_start(t[:], xp8[:, o : o + 4 * PAN])
                            x8_ch.append(t)
                        x_ch = []
                        for c in range(4):
                            t = panels.tile([P, 4, PAN], bf16, tag="panel",
                                            name=f"x_sb{n}_{c}")
                            o = (n * 16 + c * 4) * PAN
                            (nc.sync if c % 2 == 0 else nc.scalar).dma_start(
                                t[:], xp[:, o : o + 4 * PAN])
                            x_ch.append(t)
                    gin8 = f8rows(ag_in[n])
                    # c_q in fp8 DoubleRow: kt-pair-major over 3 concurrent
                    # psums so matmuls start as soon as the first chunk lands
                    pss = [psA.tile([P, PAN], f32, tag="psA", name=f"psA{n}_{m}")
                           for m in range(4)]
                    for j in range(KT_D // 2):
                        xk = x8_ch[j // 2][:, j % 2, :, :]
                        st0, st1 = (j == 0), (j == KT_D // 2 - 1)
                        for m in range(3):
                            nc.tensor.matmul(
                                pss[m],
                                lhsT=wqd_sb[:, j, :, m * P : (m + 1) * P],
                                rhs=xk, start=st0, stop=st1, perf_mode=DR,
                            )
                    # c_kv (bf16, feeds v) + local k_rope share the x chunks
                    ps_kr = psA.tile([64, PAN], f32, tag="psKR", bufs=1)
                    for kt in range(KT_D):
                        xk = x_ch[kt // 4][:, kt % 4, :]
                        st0, st1 = (kt == 0), (kt == KT_D - 1)
                        nc.tensor.matmul(pss[3], lhsT=wkvd_sb[:, kt, :],
                                         rhs=xk, start=st0, stop=st1)
                        nc.tensor.matmul(ps_kr, lhsT=wkr_sb[:, kt, :],
                                         rhs=xk, start=st0, stop=st1)
                    cqr = gin8[0:384, :].rearrange("(p r) s -> p r s", r=3)
                    for m in range(3):  # c_q -> fp8 staging (scale 1/64 back)
                        st = work.tile([P, PAN], f8, tag="cq8")
                        nc.scalar.activation(st, pss[m], AF.Copy, scale=INV64)
                        nc.sync.dma_start(cqr[:, m, :], st)
                    st = work.tile([P, PAN], bf16, tag="ckvst")
                    nc.scalar.activation(st, pss[3], AF.Copy)
                    nc.sync.dma_start(ag_in[n][192:320, :], st)
                    nc.gpsimd.collective_compute(
                        "AllGather", mybir.AluOpType.bypass,
                        replica_groups=GROUPS,
                        ins=[ag_in[n].opt()], outs=[ag_out[n].opt()],
                    )
                    krb = work.tile([64, PAN], bf16, tag="krb")
                    nc.scalar.activation(krb, ps_kr, AF.Copy)
                    rope_block(k_sb[0:64, 0, 1, ns], krb, ns, 0)
                    for hh in range(1, HL):  # replicate to the other heads
                        nc.vector.tensor_copy(k_sb[0:64, hh, 1, ns],
                                              k_sb[0:64, 0, 1, ns])

                def phase_b(n):
                    """up-projections for panel n from the gathered latents"""
                    ns = slice(n * PAN, (n + 1) * PAN)
                    gout8 = f8rows(ag_out[n])
                    cqf = bq.tile([P, 12, PAN], f8, tag="cqf")
                    nc.scalar.dma_start(
                        cqf[:],
                        gout8.rearrange("(gi x) s -> gi x s", x=2 * GR)[
                            :, 0:QSH, :
                        ].rearrange("gi (p r) s -> p gi r s", r=3),
                    )
                    ckv = bq.tile([P, KT_KV, PAN], bf16, tag="ckv")
                    nc.scalar.dma_start(
                        ckv[:],
                        ag_out[n].rearrange("(gi x) s -> gi x s", x=GR)[
                            :, 192:320, :
                        ].rearrange("gi p s -> p gi s"),
                    )
                    for m in range(HL):  # q nope heads (fp8 DoubleRow)
                        ps = psB.tile([P, PAN], f32, tag="psB")
                        for j in range(6):
                            nc.tensor.matmul(
                                ps,
                                lhsT=wqf_sb[:, j, :, m * P : (m + 1) * P],
                                rhs=cqf[:, 2 * j : 2 * j + 2, :],
                                start=(j == 0), stop=(j == 5), perf_mode=DR,
                            )
                        nc.scalar.activation(q_sb[:, m, 0, ns], ps, AF.Copy,
                                             scale=INV64)
                    for hp in range(HL // 2):  # rope: two heads per psum
                        c0 = 512 + 128 * hp
                        ps = psB.tile([P, PAN], f32, tag="psB")
                        for j in range(6):
                            nc.tensor.matmul(
                                ps,
                                lhsT=wqf_sb[:, j, :, c0 : c0 + 128],
                                rhs=cqf[:, 2 * j : 2 * j + 2, :],
                                start=(j == 0), stop=(j == 5), perf_mode=DR,
                            )
                        qrp = work.tile([P, PAN], bf16, tag="qrp")
                        nc.scalar.activation(qrp, ps, AF.Copy, scale=INV64)
                        rope_block(q_sb[0:64, 2 * hp, 1, ns], qrp[0:64, :], ns, 0)
                        rope_block(q_sb[0:64, 2 * hp + 1, 1, ns],
                                   qrp[64:128, :], ns, 64)
                    for m in range(HL):  # k_c
                        ps = psB.tile([P, PAN], f32, tag="psB")
                        for kt in range(KT_KV):
                            nc.tensor.matmul(
                                ps,
                                lhsT=wku_sb[:, kt, m * P : (m + 1) * P],
                                rhs=ckv[:, kt, :],
                                start=(kt == 0), stop=(kt == KT_KV - 1),
                            )
                        nc.scalar.activation(k_sb[:, m, 0, ns], ps, AF.Copy)
                    for sti in range(4):  # v for this panel's S-tiles
                        ps = psB.tile([P, PAN], f32, tag="psB")
                        for kt in range(KT_KV):
                            nc.tensor.matmul(
                                ps,
                                lhsT=ckv[:, kt, sti * P : (sti + 1) * P],
                                rhs=wvu_sb[:, kt, :],
                                start=(kt == 0), stop=(kt == KT_KV - 1),
                            )
                        nc.scalar.activation(v_sb[:, 4 * n + sti, :], ps, AF.Copy)

                phase_a(0)
                phase_a(1)
                phase_a(2)
                phase_b(0)
                phase_a(3)
                phase_b(1)
                phase_b(2)
                phase_b(3)

            # ---------------- Phase C: SDPA + Phase D interleaved --------
            if "C" not in phases:
                # timing-partial build: consume B outputs so nothing is elided
                nc.gpsimd.dma_start(y[0:P, 0:PAN], q_sb[:, 0, 0, 0:PAN])
                nc.gpsimd.dma_start(y[P : 2 * P, 0:PAN], k_sb[:, 0, 0, 0:PAN])
                nc.gpsimd.dma_start(y[2 * P : 3 * P, 0:PAN], v_sb[:, 0, 0:PAN])
            elif True:
                do_cd(tc, nc, phases, q_sb, k_sb, v_sb, ao_sb, masks_sb,
                      onc_sb, wo, work, y)

    nc.compile()
    return nc


def do_cd(tc, nc, phases, q_sb, k_sb, v_sb, ao_sb, masks_sb, onc_sb, wo,
          work, y):
    import concourse.mybir as mybir

    dt = mybir.dt
    f32, bf16 = dt.float32, dt.bfloat16
    AF = mybir.ActivationFunctionType
    DR = mybir.MatmulPerfMode.DoubleRow
    if True:
        if True:
            with (
                tc.tile_pool(name="pw", bufs=1) as pw,
                tc.tile_pool(name="pe", bufs=10) as pe,
                tc.tile_pool(name="pds", bufs=2) as pds,
                tc.tile_pool(name="py", bufs=4) as py,
                tc.tile_pool(name="psS", bufs=2, space="PSUM") as psS,
                tc.tile_pool(name="psO", bufs=2, space="PSUM") as psO,
                tc.tile_pool(name="psDn", bufs=1, space="PSUM") as psDn,
                tc.tile_pool(name="psD", bufs=3, space="PSUM") as psD,
            ):
                wo_sb = pw.tile([P, HL, D], bf16, tag="wo")
                nc.gpsimd.dma_start(wo_sb[:], wo[:])
                ycnt = [0]

                def d_unit(m, nh, tail=False):
                    """one output-projection unit: 2 nn-blocks of row-tile m"""
                    ms = slice(m * P, (m + 1) * P)
                    yst = py.tile([P, 2 * PAN], bf16, tag="yst")
                    for sub in range(2):
                        nn = 2 * nh + sub
                        ps = psD.tile([P, PAN], f32, tag="psD")
                        for kt in range(HL):
                            nc.tensor.matmul(
                                ps,
                                lhsT=ao_sb[:, kt, ms],
                                rhs=wo_sb[:, kt, nn * PAN : (nn + 1) * PAN],
                                start=(kt == 0), stop=(kt == HL - 1),
                            )
                        nc.vector.tensor_copy(
                            yst[:, sub * PAN : (sub + 1) * PAN], ps)
                    q = (nc.sync, nc.gpsimd, nc.scalar)[ycnt[0] % 3]
                    ycnt[0] += 1
                    q.dma_start(y[ms, 2 * nh * PAN : (2 * nh + 2) * PAN], yst)

                dq = []  # pending D units, drained inside the SDPA stream

                def d_fill():
                    if dq and "D" in phases:
                        m, nh = dq.pop(0)
                        d_unit(m, nh)

                def sdpa_pair(g, h0):
                    """two heads' SDPA chains interleaved kb-by-kb so the
                    in-order PE queue always has independent ready work while
                    the other chain waits on its exp"""
                    gs = slice(g * PAN, (g + 1) * PAN)
                    nk = 4 * (g + 1)
                    hs = (h0, h0 + 1)
                    ps_o = {h: psO.tile([P, PAN], f32, tag="ps_o",
                                        name=f"pso{g}_{h}") for h in hs}
                    # both chains' denominators live in one psum bank, at
                    # partition bases 0 and 32 (legal matmul tile positions)
                    ps_d2 = psDn.tile([33, PAN], f32, tag="ps_d")
                    e_t = {h: [None] * nk for h in hs}
                    dsum = {h: [None] * (nk // 2) for h in hs}
                    qsum = {h: [None] * (nk // 4) for h in hs}
                    pend = {h: [] for h in hs}

                    def consume(ci, h, kb):
                        nc.tensor.matmul(
                            ps_o[h], lhsT=v_sb[:, kb, h * P : (h + 1) * P],
                            rhs=e_t[h][kb],
                            start=(kb == 0), stop=(kb == nk - 1),
                        )
                        if kb % 4 == 3:
                            nc.tensor.matmul(
                                ps_d2[32 * ci : 32 * ci + 1, :],
                                lhsT=onc_sb[:], rhs=qsum[h][kb // 4],
                                start=(kb == 3), stop=(kb == nk - 1),
                                skip_group_check=True,
                            )

                    for kb in range(nk):
                        ks = slice(kb * P, (kb + 1) * P)
                        for ci, h in enumerate(hs):
                            ps_s = psS.tile([P, PAN], f32, tag="ps_s")
                            nc.tensor.matmul(
                                ps_s, lhsT=k_sb[:, h, :, ks],
                                rhs=q_sb[:, h, :, gs],
                                start=True, stop=True, perf_mode=DR,
                            )
                            e = pe.tile([P, PAN], bf16, tag="e")
                            nc.scalar.activation(e, ps_s, AF.Exp)
                            if kb >= 4 * g:
                                nc.vector.tensor_mul(
                                    e, e, masks_sb[:, kb - 4 * g, :])
                            e_t[h][kb] = e
                            if kb % 2 == 1:
                                ds = pds.tile([P, PAN], bf16, tag=f"dsum{ci}")
                                nc.vector.tensor_add(ds, e_t[h][kb - 1],
                                                     e_t[h][kb])
                                dsum[h][kb // 2] = ds
                            if kb % 4 == 3:
                                qs = pds.tile([P, PAN], bf16, tag=f"qsum{ci}")
                                nc.vector.tensor_add(qs, dsum[h][kb // 2 - 1],
                                                     dsum[h][kb // 2])
                                qsum[h][kb // 4] = qs
                            pend[h].append(kb)
                            if len(pend[h]) > 3:
                                consume(ci, h, pend[h].pop(0))
                    for ci, h in enumerate(hs):
                        while pend[h]:
                            consume(ci, h, pend[h].pop(0))
                    for ci, h in enumerate(hs):
                        rc = work.tile([1, PAN], f32, tag="rc")
                        nc.vector.reciprocal(rc, ps_d2[32 * ci : 32 * ci + 1, :])
                        bb = work.tile([P, PAN], f32, tag="bb")
                        nc.gpsimd.partition_broadcast(bb, rc)
                        nc.vector.tensor_mul(ao_sb[:, h, gs], ps_o[h], bb)

                for g in range(G):
                    if g >= 1:
                        dq.extend((4 * (g - 1) + mm, nh)
                                  for mm in range(4) for nh in range(2))
                    for hp in range(HL // 2):
                        sdpa_pair(g, 2 * hp)
                        for _ in range(4):
                            d_fill()
                    while dq:
                        d_fill()
                if "D" in phases:
                    for m in range(4 * (G - 1), 4 * G):
                        for nh in range(2):
                            d_unit(m, nh, tail=True)


def _prep_inputs(x, positions, Wq_down, Wq_up, Wq_rope, Wkv_down, Wk_up, Wv_up,
                 Wk_rope, Wo):
    s2 = np.float32(QK_D ** -0.25)  # sqrt of the 1/sqrt(qk_d) scale
    bf = lambda a: np.ascontiguousarray(a).astype(BF16)

    def pmaj(a, kt):
        # [kt*128, m] -> [128, kt*m] partition-major
        m = a.shape[1]
        return np.ascontiguousarray(
            a.reshape(kt, P, m).transpose(1, 0, 2).reshape(P, kt * m))

    inv_freq = 1.0 / (10000.0 ** (np.arange(0, RD, 2, dtype=np.float32) / RD))
    ang = positions.astype(np.float32)[:, None] * inv_freq  # (S, 32)
    shared = {
        "wkr": pmaj(Wk_rope.T * s2, 16).astype(BF16),
        "onc": np.ones((P, 1), BF16),
        "cosb": bf(np.tile(np.cos(ang).T, (4, 1))),
        "sinb": bf(np.tile(np.sin(ang).T, (4, 1))),
    }

    mk = np.zeros((P, G * PAN), np.float32)
    for j in range(G):
        p = np.arange(P)[:, None]
        q = np.arange(PAN)[None, :]
        mk[:, j * PAN : (j + 1) * PAN] = (j * P + p <= q).astype(np.float32)
    shared["masks"] = mk.astype(BF16)

    wqdT = Wq_down.T  # (D, QR)
    wkvdT = Wkv_down.T  # (D, KVR)
    per_g = []
    for g in range(G):
        rs, rr = slice(512 * g, 512 * (g + 1)), slice(256 * g, 256 * (g + 1))
        wqall = np.concatenate([Wq_up[rs].T, Wq_rope[rr].T], axis=1)
        per_g.append({
            "wqd8": pmaj(wqdT[:, QSH * g : QSH * (g + 1)] * 64.0, 16).astype(F8),
            "wkvd": pmaj(wkvdT[:, KSH * g : KSH * (g + 1)], 16).astype(BF16),
            "wqf": pmaj(wqall * (s2 * 64.0), 12).astype(F8),
            "wku": pmaj(Wk_up[rs].T * s2, 4).astype(BF16),
            "wvu": pmaj(Wv_up[rs].T, 4).astype(BF16),
            "wo": pmaj(Wo[:, rs].T, 4).astype(BF16),
        })
    # x as [p, panel, kt, s] partition-major flat
    def xprep(a):
        # a: [D, S] -> [128, NP*16*PAN]
        return np.ascontiguousarray(
            a.reshape(16, P, 4, PAN).transpose(1, 2, 0, 3).reshape(P, -1))

    xTs = [x[b].T for b in range(B)]
    xps = [xprep(a).astype(BF16) for a in xTs]
    xp8s = [xprep(a).astype(F8) for a in xTs]

    in_maps = []
    for c in range(8):
        b, g = c // G, c % G
        m = dict(shared)
        m.update(per_g[g])
        m["xp"] = xps[b]
        m["xp8"] = xp8s[b]
        in_maps.append(m)
    return in_maps


def kernel(**inputs):
    from concourse.bass_utils import run_bass_kernel_spmd

    if "nc" not in _cache:
        _cache["nc"] = _build_module()
    nc = _cache["nc"]

    in_maps = _prep_inputs(**inputs)
    res = None
    for attempt in range(3):
        try:
            res = run_bass_kernel_spmd(nc, in_maps, core_ids=list(range(8)))
            break
        except Exception:
            if attempt == 2:
                raise
    out = np.zeros((B, S, D), np.float32)
    for c in range(8):
        out[c // G] += res.results[c]["y"].astype(np.float32)
    return out


# revision 84
# speedup vs baseline: 1.0051x; 1.0018x over previous
"""Trainium2 Bass kernel for MultiLatentAttention (MLA).

Sharding: 8 cores = 2 (batch) x 4 (head-groups of 4 heads).
Within each batch group of 4 cores, the down-projections are sharded by
output rows and AllGathered (per S-panel, pipelined).  Each core computes
the shared k_rope head locally for all panels (no collective), runs its
4 heads' up-projections + SDPA and a partial output projection
y_part = attn_out @ Wo[:, heads].T.  Host sums the 4 partials per batch.

Speed tricks vs the bf16 baseline:
- Scores are computed with fp8e4 DoubleRow matmuls: the 192-dim qk
  contraction (128 nope + 64 rope) is packed as [128, 2, *] operands and
  runs at 0.5 cycles/column (4x fewer PE cycles than 2 bf16 matmuls).
- The q up-projection also runs in fp8 DoubleRow (k-tile pairs), with
  the fp8 weights pre-scaled by 64 to stay in e4m3 normal range; the 64
  is divided back out in the psum->q cast on the Act engine.
- c_q travels through the AllGather in fp8 (mixed fp8/bf16 gather buffer
  via bitcast views), shrinking collective bytes and killing the
  readback cast.
- Softmax denominators use DVE pair-sums of e-tiles, halving the
  ones-matmul count; attention output stays in SBUF (no DRAM roundtrip)
  and the output projection is software-pipelined into the SDPA stream.
- 1/sqrt(192) is split as s2 = 192**-0.25 folded into both the q-side
  and k-side weights so fp8 operands are magnitude-balanced.
All fp8/bf16 matmuls accumulate in f32 PSUM; y partials are f32.
"""

import sys

if "/opt/trn_rl_repo" not in sys.path:
    sys.path.insert(0, "/opt/trn_rl_repo")

import numpy as np
import ml_dtypes

BF16 = ml_dtypes.bfloat16
F8 = ml_dtypes.float8_e4m3

B, S, D, H = 2, 2048, 2048, 16
QR, KVR = 1536, 512
NOPE, RD, VD = 128, 64, 128
QK_D = NOPE + RD
HL = 4          # heads per core
G = 4           # head groups (= cores per batch group)
QSH = QR // G   # 384 c_q rows per core
KSH = KVR // G  # 128 c_kv rows per core
PAN = 512       # panel width
P = 128
GR = 320        # gather rows (bf16 units): 192 (cq fp8) + 128 (ckv bf16)

_cache = {}


def _build_module(phases="ABCD"):
    import concourse.bacc as bacc
    import concourse.mybir as mybir
    import concourse.tile as tile

    dt = mybir.dt
    f32, bf16, f8 = dt.float32, dt.bfloat16, dt.float8e4
    AF = mybir.ActivationFunctionType
    DR = mybir.MatmulPerfMode.DoubleRow

    nc = bacc.Bacc("TRN2", target_bir_lowering=False, debug=False, num_devices=8)

    def inp(name, shape, dtype=bf16):
        return nc.dram_tensor(name, shape, dtype, kind="ExternalInput").ap()

    # all inputs arrive pre-arranged partition-major ([128, flat]) so every
    # load is a contiguous 2-D DMA (128 descriptors, no strided gather)
    xp = inp("xp", [P, 4 * 16 * PAN])       # x[b].T as [p, panel, kt, s]
    xp8 = inp("xp8", [P, 4 * 16 * PAN], f8)
    wqd8 = inp("wqd8", [P, 8 * 2 * QSH], f8)   # Wq_down.T slice * 64, [p,j,two,m]
    wkvd = inp("wkvd", [P, 16 * KSH])       # Wkv_down.T slice, [p,kt,m]
    wkr = inp("wkr", [P, 16 * RD])          # Wk_rope.T * s2, [p,kt,m]
    wqf = inp("wqf", [P, 6 * 2 * 768], f8)  # [Wq_up|Wq_rope].T * s2*64
    wku = inp("wku", [P, 4 * 512])          # Wk_up_g.T * s2, [p,kt,m]
    wvu = inp("wvu", [P, 4 * 512])          # Wv_up_g.T
    wo = inp("wo", [P, 4 * D])              # Wo[:, cols_g].T, [p,kt,m]
    cosb = inp("cosb", [P, S])              # bf16 rope tables, 4x partition-tiled
    sinb = inp("sinb", [P, S])
    masks = inp("masks", [P, G * PAN])      # multiplicative causal masks
    onc = inp("onc", [P, 1])                # ones column
    y = nc.dram_tensor("y", [S, D], bf16, kind="ExternalOutput").ap()

    KT_D = D // P      # 16 k-tiles over model dim
    KT_KV = KVR // P   # 4
    NP = S // PAN      # 4 panels
    GROUPS = [[0, 1, 2, 3], [4, 5, 6, 7]]
    INV64 = 1.0 / 64.0

    with tile.TileContext(nc) as tc:
        with (
            tc.tile_pool(name="res", bufs=1) as res,
            tc.tile_pool(name="panels", bufs=8) as panels,
            tc.tile_pool(name="work", bufs=2) as work,
            tc.tile_pool(name="dram", bufs=1, space="DRAM") as dram,
        ):
            # ---- SBUF residents ----------------------------------------
            # q/k fp8 layout [P, head, slot, S]: slot 0 = nope dims,
            # slot 1 rows 0:64 = roped rope dims, rows 64:128 zero.
            q_sb = res.tile([P, HL, 2, S], f8, tag="q")
            k_sb = res.tile([P, HL, 2, S], f8, tag="k")
            v_sb = res.tile([P, S // P, 512], bf16, tag="v")
            ao_sb = res.tile([P, HL, S], bf16, tag="ao")
            masks_sb = res.tile([P, G, PAN], bf16, tag="masks")
            onc_sb = res.tile([P, 1], bf16, tag="onc")
            cos_sb = res.tile([P, S], bf16, tag="cos")
            sin_sb = res.tile([P, S], bf16, tag="sin")

            # zero the unused upper rope rows so the fp8 matmul never sees
            # garbage (e4m3 NaN patterns would poison 0*NaN)
            nc.vector.memset(q_sb[64:128, :, 1, :], 0.0)
            nc.vector.memset(k_sb[64:128, :, 1, :], 0.0)
            nc.gpsimd.dma_start(
                masks_sb[:], masks.rearrange("p (j q) -> p j q", q=PAN)
            )
            nc.gpsimd.dma_start(onc_sb[:], onc[:])
            nc.gpsimd.dma_start(cos_sb[:], cosb[:])
            nc.gpsimd.dma_start(sin_sb[:], sinb[:])

            # ---- DRAM staging for the latent AllGather ------------------
            ag_in = [dram.tile([GR, PAN], bf16, tag=f"agi{n}", name=f"agi{n}")
                     for n in range(NP)]
            ag_out = [dram.tile([G * GR, PAN], bf16, tag=f"ago{n}",
                                name=f"ago{n}") for n in range(NP)]

            def f8rows(t):
                # [rows, PAN] bf16 -> [2*rows, PAN] fp8 view of same bytes
                return t[:].bitcast(f8).rearrange("r (two s) -> (r two) s",
                                                  s=PAN)

            def rope_block(dst64, src64, ns, b0):
                # dst [64, PAN] fp8 slot-1 rows; src [64, PAN] bf16 whose
                # halves sit at absolute partitions b0 and b0+32 (the BIR
                # verifier wants matching base partitions for SBUF inputs,
                # hence the partition-tiled cos/sin tables)
                cs0, cs1 = cos_sb[b0 : b0 + 32, ns], cos_sb[b0 + 32 : b0 + 64, ns]
                sn0, sn1 = sin_sb[b0 : b0 + 32, ns], sin_sb[b0 + 32 : b0 + 64, ns]
                t1 = work.tile([32, PAN], bf16, tag="rt1")
                t2 = work.tile([32, PAN], bf16, tag="rt2")
                nc.vector.tensor_mul(t1, src64[0:32, :], cs0)
                nc.vector.tensor_mul(t2, src64[32:64, :], sn1)
                nc.vector.tensor_sub(dst64[0:32, :], t1, t2)
                t3 = work.tile([32, PAN], bf16, tag="rt1")
                t4 = work.tile([32, PAN], bf16, tag="rt2")
                nc.vector.tensor_mul(t3, src64[32:64, :], cs1)
                nc.vector.tensor_mul(t4, src64[0:32, :], sn0)
                nc.vector.tensor_add(dst64[32:64, :], t3, t4)

            # ---- Phase A + B -------------------------------------------
            with (
                tc.tile_pool(name="pa", bufs=1) as pa,
                tc.tile_pool(name="pb", bufs=1) as pb,
                tc.tile_pool(name="bq", bufs=3) as bq,
                tc.tile_pool(name="psA", bufs=4, space="PSUM") as psA,
                tc.tile_pool(name="psB", bufs=3, space="PSUM") as psB,
            ):
                # A weights: wqd8 chunked so the first matmul starts early
                wqd_sb = pa.tile([P, KT_D // 2, 2, QSH], f8, tag="wqd8")
                x80_ch = []
                for c in range(4):
                    nc.sync.dma_start(wqd_sb[:, 2 * c : 2 * c + 2, :, :],
                                      wqd8[:, c * 2 * 2 * QSH : (c + 1) * 2 * 2 * QSH])
                    t = panels.tile([P, 2, 2, PAN], f8, tag="panel8",
                                    name=f"x8_sb0_{c}")
                    nc.sync.dma_start(t[:], xp8[:, c * 4 * PAN : (c + 1) * 4 * PAN])
                    x80_ch.append(t)
                wkvd_sb = pa.tile([P, KT_D, KSH], bf16, tag="wkvd")
                nc.sync.dma_start(wkvd_sb[:], wkvd[:])
                wkr_sb = pa.tile([P, KT_D, RD], bf16, tag="wkr")
                nc.sync.dma_start(wkr_sb[:], wkr[:])
                x0_ch = []
                for c in range(4):
                    t = panels.tile([P, 4, PAN], bf16, tag="panel",
                                    name=f"x_sb0_{c}")
                    (nc.sync if c % 2 == 0 else nc.scalar).dma_start(
                        t[:], xp[:, c * 4 * PAN : (c + 1) * 4 * PAN])
                    x0_ch.append(t)

                # B weights on the gpsimd DMA queue, off the hot SP queue
                def load_b_weights():
                    wqf_sb = pb.tile([P, 6, 2, 768], f8, tag="wqf")
                    nc.gpsimd.dma_start(wqf_sb[:], wqf[:])
                    wku_sb = pb.tile([P, KT_KV, 512], bf16, tag="wku")
                    nc.gpsimd.dma_start(wku_sb[:], wku[:])
                    wvu_sb = pb.tile([P, KT_KV, 512], bf16, tag="wvu")
                    nc.gpsimd.dma_start(wvu_sb[:], wvu[:])
                    return wqf_sb, wku_sb, wvu_sb

                def phase_a(n):
                    """my latent slices for panel n + local k_rope, then gather"""
                    ns = slice(n * PAN, (n + 1) * PAN)
                    if n == 0:
                        x_ch, x8_ch = x0_ch, x80_ch
                    else:
                        x8_ch = []
                        for c in range(4):
                            t = panels.tile([P, 2, 2, PAN], f8, tag="panel8",
                                            name=f"x8_sb{n}_{c}")
                            o = (n * 16 + c * 4) * PAN
                            nc.sync.dma_start(t[:], xp8[:, o : o + 4 * PAN])
                            x8_ch.append(t)
                        x_ch = []
                        for c in range(4):
                            t = panels.tile([P, 4, PAN], bf16, tag="panel",
                                            name=f"x_sb{n}_{c}")
                            o = (n * 16 + c * 4) * PAN
                            (nc.sync if c % 2 == 0 else nc.scalar).dma_start(
                                t[:], xp[:, o : o + 4 * PAN])
                            x_ch.append(t)
                    gin8 = f8rows(ag_in[n])
                    # c_q in fp8 DoubleRow: kt-pair-major over 3 concurrent
                    # psums so matmuls start as soon as the first chunk lands
                    pss = [psA.tile([P, PAN], f32, tag="psA", name=f"psA{n}_{m}")
                           for m in range(4)]
                    for j in range(KT_D // 2):
                        xk = x8_ch[j // 2][:, j % 2, :, :]
                        st0, st1 = (j == 0), (j == KT_D // 2 - 1)
                        for m in range(3):
                            nc.tensor.matmul(
                                pss[m],
                                lhsT=wqd_sb[:, j, :, m * P : (m + 1) * P],
                                rhs=xk, start=st0, stop=st1, perf_mode=DR,
                            )
                    # c_kv (bf16, feeds v) + local k_rope share the x chunks
                    ps_kr = psA.tile([64, PAN], f32, tag="psKR", bufs=1)
                    for kt in range(KT_D):
                        xk = x_ch[kt // 4][:, kt % 4, :]
                        st0, st1 = (kt == 0), (kt == KT_D - 1)
                        nc.tensor.matmul(pss[3], lhsT=wkvd_sb[:, kt, :],
                                         rhs=xk, start=st0, stop=st1)
                        nc.tensor.matmul(ps_kr, lhsT=wkr_sb[:, kt, :],
                                         rhs=xk, start=st0, stop=st1)
                    cqr = gin8[0:384, :].rearrange("(p r) s -> p r s", r=3)
                    for m in range(3):  # c_q -> fp8 staging (scale 1/64 back)
                        st = work.tile([P, PAN], f8, tag="cq8")
                        nc.scalar.activation(st, pss[m], AF.Copy, scale=INV64)
                        nc.sync.dma_start(cqr[:, m, :], st)
                    st = work.tile([P, PAN], bf16, tag="ckvst")
                    nc.scalar.activation(st, pss[3], AF.Copy)
                    nc.sync.dma_start(ag_in[n][192:320, :], st)
                    nc.gpsimd.collective_compute(
                        "AllGather", mybir.AluOpType.bypass,
                        replica_groups=GROUPS,
                        ins=[ag_in[n].opt()], outs=[ag_out[n].opt()],
                    )
                    krb = work.tile([64, PAN], bf16, tag="krb")
                    nc.scalar.activation(krb, ps_kr, AF.Copy)
                    rope_block(k_sb[0:64, 0, 1, ns], krb, ns, 0)
                    for hh in range(1, HL):  # replicate to the other heads
                        nc.vector.tensor_copy(k_sb[0:64, hh, 1, ns],
                                              k_sb[0:64, 0, 1, ns])

                def phase_b(n):
                    """up-projections for panel n from the gathered latents"""
                    ns = slice(n * PAN, (n + 1) * PAN)
                    gout8 = f8rows(ag_out[n])
                    cqf = bq.tile([P, 12, PAN], f8, tag="cqf")
                    nc.scalar.dma_start(
                        cqf[:],
                        gout8.rearrange("(gi x) s -> gi x s", x=2 * GR)[
                            :, 0:QSH, :
                        ].rearrange("gi (p r) s -> p gi r s", r=3),
                    )
                    ckv = bq.tile([P, KT_KV, PAN], bf16, tag="ckv")
                    nc.scalar.dma_start(
                        ckv[:],
                        ag_out[n].rearrange("(gi x) s -> gi x s", x=GR)[
                            :, 192:320, :
                        ].rearrange("gi p s -> p gi s"),
                    )
                    for m in range(HL):  # q nope heads (fp8 DoubleRow)
                        ps = psB.tile([P, PAN], f32, tag="psB")
                        for j in range(6):
                            nc.tensor.matmul(
                                ps,
                                lhsT=wqf_sb[:, j, :, m * P : (m + 1) * P],
                                rhs=cqf[:, 2 * j : 2 * j + 2, :],
                                start=(j == 0), stop=(j == 5), perf_mode=DR,
                            )
                        nc.scalar.activation(q_sb[:, m, 0, ns], ps, AF.Copy,
                                             scale=INV64)
                    for hp in range(HL // 2):  # rope: two heads per psum
                        c0 = 512 + 128 * hp
                        ps = psB.tile([P, PAN], f32, tag="psB")
                        for j in range(6):
                            nc.tensor.matmul(
                                ps,
                                lhsT=wqf_sb[:, j, :, c0 : c0 + 128],
                                rhs=cqf[:, 2 * j : 2 * j + 2, :],
                                start=(j == 0), stop=(j == 5), perf_mode=DR,
                            )
                        qrp = work.tile([P, PAN], bf16, tag="qrp")
                        nc.scalar.activation(qrp, ps, AF.Copy, scale=INV64)
                        rope_block(q_sb[0:64, 2 * hp, 1, ns], qrp[0:64, :], ns, 0)
                        rope_block(q_sb[0:64, 2 * hp + 1, 1, ns],
                                   qrp[64:128, :], ns, 64)
                    for m in range(HL):  # k_c
                        ps = psB.tile([P, PAN], f32, tag="psB")
                        for kt in range(KT_KV):
                            nc.tensor.matmul(
                                ps,
                                lhsT=wku_sb[:, kt, m * P : (m + 1) * P],
                                rhs=ckv[:, kt, :],
                                start=(kt == 0), stop=(kt == KT_KV - 1),
                            )
                        nc.scalar.activation(k_sb[:, m, 0, ns], ps, AF.Copy)
                    for sti in range(4):  # v for this panel's S-tiles
                        ps = psB.tile([P, PAN], f32, tag="psB")
                        for kt in range(KT_KV):
                            nc.tensor.matmul(
                                ps,
                                lhsT=ckv[:, kt, sti * P : (sti + 1) * P],
                                rhs=wvu_sb[:, kt, :],
                                start=(kt == 0), stop=(kt == KT_KV - 1),
                            )
                        nc.scalar.activation(v_sb[:, 4 * n + sti, :], ps, AF.Copy)

                phase_a(0)
                wqf_sb, wku_sb, wvu_sb = load_b_weights()
                phase_a(1)
                phase_a(2)
                phase_b(0)
                phase_a(3)
                phase_b(1)
                phase_b(2)
                phase_b(3)

            # ---------------- Phase C: SDPA + Phase D interleaved --------
            if "C" not in phases:
                # timing-partial build: consume B outputs so nothing is elided
                nc.gpsimd.dma_start(y[0:P, 0:PAN], q_sb[:, 0, 0, 0:PAN])
                nc.gpsimd.dma_start(y[P : 2 * P, 0:PAN], k_sb[:, 0, 0, 0:PAN])
                nc.gpsimd.dma_start(y[2 * P : 3 * P, 0:PAN], v_sb[:, 0, 0:PAN])
            elif True:
                do_cd(tc, nc, phases, q_sb, k_sb, v_sb, ao_sb, masks_sb,
                      onc_sb, wo, work, y)

    nc.compile()
    return nc


def do_cd(tc, nc, phases, q_sb, k_sb, v_sb, ao_sb, masks_sb, onc_sb, wo,
          work, y):
    import concourse.mybir as mybir

    dt = mybir.dt
    f32, bf16 = dt.float32, dt.bfloat16
    AF = mybir.ActivationFunctionType
    DR = mybir.MatmulPerfMode.DoubleRow
    if True:
        if True:
            with (
                tc.tile_pool(name="pw", bufs=1) as pw,
                tc.tile_pool(name="pe", bufs=10) as pe,
                tc.tile_pool(name="pds", bufs=2) as pds,
                tc.tile_pool(name="py", bufs=4) as py,
                tc.tile_pool(name="psS", bufs=2, space="PSUM") as psS,
                tc.tile_pool(name="psO", bufs=2, space="PSUM") as psO,
                tc.tile_pool(name="psDn", bufs=1, space="PSUM") as psDn,
                tc.tile_pool(name="psD", bufs=3, space="PSUM") as psD,
            ):
                wo_sb = pw.tile([P, HL, D], bf16, tag="wo")
                nc.gpsimd.dma_start(wo_sb[:], wo[:])
                ycnt = [0]

                def d_unit(m, nh, tail=False):
                    """one output-projection unit: 2 nn-blocks of row-tile m"""
                    ms = slice(m * P, (m + 1) * P)
                    yst = py.tile([P, 2 * PAN], bf16, tag="yst")
                    for sub in range(2):
                        nn = 2 * nh + sub
                        ps = psD.tile([P, PAN], f32, tag="psD")
                        for kt in range(HL):
                            nc.tensor.matmul(
                                ps,
                                lhsT=ao_sb[:, kt, ms],
                                rhs=wo_sb[:, kt, nn * PAN : (nn + 1) * PAN],
                                start=(kt == 0), stop=(kt == HL - 1),
                            )
                        nc.vector.tensor_copy(
                            yst[:, sub * PAN : (sub + 1) * PAN], ps)
                    q = (nc.sync, nc.gpsimd, nc.scalar)[ycnt[0] % 3]
                    ycnt[0] += 1
                    q.dma_start(y[ms, 2 * nh * PAN : (2 * nh + 2) * PAN], yst)

                dq = []  # pending D units, drained inside the SDPA stream

                def d_fill():
                    if dq and "D" in phases:
                        m, nh = dq.pop(0)
                        d_unit(m, nh)

                def sdpa_pair(g, h0):
                    """two heads' SDPA chains interleaved kb-by-kb so the
                    in-order PE queue always has independent ready work while
                    the other chain waits on its exp"""
                    gs = slice(g * PAN, (g + 1) * PAN)
                    nk = 4 * (g + 1)
                    hs = (h0, h0 + 1)
                    ps_o = {h: psO.tile([P, PAN], f32, tag="ps_o",
                                        name=f"pso{g}_{h}") for h in hs}
                    # both chains' denominators live in one psum bank, at
                    # partition bases 0 and 32 (legal matmul tile positions)
                    ps_d2 = psDn.tile([33, PAN], f32, tag="ps_d")
                    e_t = {h: [None] * nk for h in hs}
                    dsum = {h: [None] * (nk // 2) for h in hs}
                    qsum = {h: [None] * (nk // 4) for h in hs}
                    pend = {h: [] for h in hs}

                    def consume(ci, h, kb):
                        nc.tensor.matmul(
                            ps_o[h], lhsT=v_sb[:, kb, h * P : (h + 1) * P],
                            rhs=e_t[h][kb],
                            start=(kb == 0), stop=(kb == nk - 1),
                        )
                        if kb % 4 == 3:
                            nc.tensor.matmul(
                                ps_d2[32 * ci : 32 * ci + 1, :],
                                lhsT=onc_sb[:], rhs=qsum[h][kb // 4],
                                start=(kb == 3), stop=(kb == nk - 1),
                                skip_group_check=True,
                            )

                    for kb in range(nk):
                        ks = slice(kb * P, (kb + 1) * P)
                        for ci, h in enumerate(hs):
                            ps_s = psS.tile([P, PAN], f32, tag="ps_s")
                            nc.tensor.matmul(
                                ps_s, lhsT=k_sb[:, h, :, ks],
                                rhs=q_sb[:, h, :, gs],
                                start=True, stop=True, perf_mode=DR,
                            )
                            e = pe.tile([P, PAN], bf16, tag="e")
                            nc.scalar.activation(e, ps_s, AF.Exp)
                            if kb >= 4 * g:
                                nc.vector.tensor_mul(
                                    e, e, masks_sb[:, kb - 4 * g, :])
                            e_t[h][kb] = e
                            if kb % 2 == 1:
                                ds = pds.tile([P, PAN], bf16, tag=f"dsum{ci}")
                                nc.vector.tensor_add(ds, e_t[h][kb - 1],
                                                     e_t[h][kb])
                                dsum[h][kb // 2] = ds
                            if kb % 4 == 3:
                                qs = pds.tile([P, PAN], bf16, tag=f"qsum{ci}")
                                nc.vector.tensor_add(qs, dsum[h][kb // 2 - 1],
                                                     dsum[h][kb // 2])
                                qsum[h][kb // 4] = qs
                            pend[h].append(kb)
                            if len(pend[h]) > 3:
                                consume(ci, h, pend[h].pop(0))
                    for ci, h in enumerate(hs):
                        while pend[h]:
                            consume(ci, h, pend[h].pop(0))
                    for ci, h in enumerate(hs):
                        rc = work.tile([1, PAN], f32, tag="rc")
                        nc.vector.reciprocal(rc, ps_d2[32 * ci : 32 * ci + 1, :])
                        bb = work.tile([P, PAN], f32, tag="bb")
                        nc.gpsimd.partition_broadcast(bb, rc)
                        nc.vector.tensor_mul(ao_sb[:, h, gs], ps_o[h], bb)

                for g in range(G):
                    if g >= 1:
                        dq.extend((4 * (g - 1) + mm, nh)
                                  for mm in range(4) for nh in range(2))
                    for hp in range(HL // 2):
                        sdpa_pair(g, 2 * hp)
                        for _ in range(4):
                            d_fill()
                    while dq:
                        d_fill()
                if "D" in phases:
                    for m in range(4 * (G - 1), 4 * G):
                        for nh in range(2):
                            d_unit(m, nh, tail=True)


def _prep_inputs(x, positions, Wq_down, Wq_up, Wq_rope, Wkv_down, Wk_up, Wv_up,
                 Wk_rope, Wo):
    s2 = np.float32(QK_D ** -0.25)  # sqrt of the 1/sqrt(qk_d) scale
    bf = lambda a: np.ascontiguousarray(a).astype(BF16)

    def pmaj(a, kt):
        # [kt*128, m] -> [128, kt*m] partition-major
        m = a.shape[1]
        return np.ascontiguousarray(
            a.reshape(kt, P, m).transpose(1, 0, 2).reshape(P, kt * m))

    inv_freq = 1.0 / (10000.0 ** (np.arange(0, RD, 2, dtype=np.float32) / RD))
    ang = positions.astype(np.float32)[:, None] * inv_freq  # (S, 32)
    shared = {
        "wkr": pmaj(Wk_rope.T * s2, 16).astype(BF16),
        "onc": np.ones((P, 1), BF16),
        "cosb": bf(np.tile(np.cos(ang).T, (4, 1))),
        "sinb": bf(np.tile(np.sin(ang).T, (4, 1))),
    }

    mk = np.zeros((P, G * PAN), np.float32)
    for j in range(G):
        p = np.arange(P)[:, None]
        q = np.arange(PAN)[None, :]
        mk[:, j * PAN : (j + 1) * PAN] = (j * P + p <= q).astype(np.float32)
    shared["masks"] = mk.astype(BF16)

    wqdT = Wq_down.T  # (D, QR)
    wkvdT = Wkv_down.T  # (D, KVR)
    per_g = []
    for g in range(G):
        rs, rr = slice(512 * g, 512 * (g + 1)), slice(256 * g, 256 * (g + 1))
        wqall = np.concatenate([Wq_up[rs].T, Wq_rope[rr].T], axis=1)
        per_g.append({
            "wqd8": pmaj(wqdT[:, QSH * g : QSH * (g + 1)] * 64.0, 16).astype(F8),
            "wkvd": pmaj(wkvdT[:, KSH * g : KSH * (g + 1)], 16).astype(BF16),
            "wqf": pmaj(wqall * (s2 * 64.0), 12).astype(F8),
            "wku": pmaj(Wk_up[rs].T * s2, 4).astype(BF16),
            "wvu": pmaj(Wv_up[rs].T, 4).astype(BF16),
            "wo": pmaj(Wo[:, rs].T, 4).astype(BF16),
        })
    # x as [p, panel, kt, s] partition-major flat
    def xprep(a):
        # a: [D, S] -> [128, NP*16*PAN]
        return np.ascontiguousarray(
            a.reshape(16, P, 4, PAN).transpose(1, 2, 0, 3).reshape(P, -1))

    xTs = [x[b].T for b in range(B)]
    xps = [xprep(a).astype(BF16) for a in xTs]
    xp8s = [xprep(a).astype(F8) for a in xTs]

    in_maps = []
    for c in range(8):
        b, g = c // G, c % G
        m = dict(shared)
        m.update(per_g[g])
        m["xp"] = xps[b]
        m["xp8"] = xp8s[b]
        in_maps.append(m)
    return in_maps


def kernel(**inputs):
    from concourse.bass_utils import run_bass_kernel_spmd

    if "nc" not in _cache:
        _cache["nc"] = _build_module()
    nc = _cache["nc"]

    in_maps = _prep_inputs(**inputs)
    res = None
    for attempt in range(3):
        try:
            res = run_bass_kernel_spmd(nc, in_maps, core_ids=list(range(8)))
            break
        except Exception:
            if attempt == 2:
                raise
    out = np.zeros((B, S, D), np.float32)
    for c in range(8):
        out[c // G] += res.results[c]["y"].astype(np.float32)
    return out


# revision 92
# speedup vs baseline: 1.0056x; 1.0006x over previous
"""Trainium2 Bass kernel for MultiLatentAttention (MLA).

Sharding: 8 cores = 2 (batch) x 4 (head-groups of 4 heads).
Within each batch group of 4 cores, the down-projections are sharded by
output rows and AllGathered (per S-panel, pipelined).  Each core computes
the shared k_rope head locally for all panels (no collective), runs its
4 heads' up-projections + SDPA and a partial output projection
y_part = attn_out @ Wo[:, heads].T.  Host sums the 4 partials per batch.

Speed tricks vs the bf16 baseline:
- Scores are computed with fp8e4 DoubleRow matmuls: the 192-dim qk
  contraction (128 nope + 64 rope) is packed as [128, 2, *] operands and
  runs at 0.5 cycles/column (4x fewer PE cycles than 2 bf16 matmuls).
- The q up-projection also runs in fp8 DoubleRow (k-tile pairs), with
  the fp8 weights pre-scaled by 64 to stay in e4m3 normal range; the 64
  is divided back out in the psum->q cast on the Act engine.
- c_q travels through the AllGather in fp8 (mixed fp8/bf16 gather buffer
  via bitcast views), shrinking collective bytes and killing the
  readback cast.
- Softmax denominators use DVE pair-sums of e-tiles, halving the
  ones-matmul count; attention output stays in SBUF (no DRAM roundtrip)
  and the output projection is software-pipelined into the SDPA stream.
- 1/sqrt(192) is split as s2 = 192**-0.25 folded into both the q-side
  and k-side weights so fp8 operands are magnitude-balanced.
All fp8/bf16 matmuls accumulate in f32 PSUM; y partials are f32.
"""

import sys

if "/opt/trn_rl_repo" not in sys.path:
    sys.path.insert(0, "/opt/trn_rl_repo")

import numpy as np
import ml_dtypes

BF16 = ml_dtypes.bfloat16
F8 = ml_dtypes.float8_e4m3

B, S, D, H = 2, 2048, 2048, 16
QR, KVR = 1536, 512
NOPE, RD, VD = 128, 64, 128
QK_D = NOPE + RD
HL = 4          # heads per core
G = 4           # head groups (= cores per batch group)
QSH = QR // G   # 384 c_q rows per core
KSH = KVR // G  # 128 c_kv rows per core
PAN = 512       # panel width
P = 128
GR = 320        # gather rows (bf16 units): 192 (cq fp8) + 128 (ckv bf16)

_cache = {}


def _build_module(phases="ABCD"):
    import concourse.bacc as bacc
    import concourse.mybir as mybir
    import concourse.tile as tile

    dt = mybir.dt
    f32, bf16, f8 = dt.float32, dt.bfloat16, dt.float8e4
    AF = mybir.ActivationFunctionType
    DR = mybir.MatmulPerfMode.DoubleRow

    nc = bacc.Bacc("TRN2", target_bir_lowering=False, debug=False, num_devices=8)

    def inp(name, shape, dtype=bf16):
        return nc.dram_tensor(name, shape, dtype, kind="ExternalInput").ap()

    # all inputs arrive pre-arranged partition-major ([128, flat]) so every
    # load is a contiguous 2-D DMA (128 descriptors, no strided gather)
    xp = inp("xp", [P, 4 * 16 * PAN])       # x[b].T as [p, panel, kt, s]
    xp8 = inp("xp8", [P, 4 * 16 * PAN], f8)
    wqd8 = inp("wqd8", [P, 8 * 2 * QSH], f8)   # Wq_down.T slice * 64, [p,j,two,m]
    wkvd = inp("wkvd", [P, 16 * KSH])       # Wkv_down.T slice, [p,kt,m]
    wkr = inp("wkr", [P, 16 * RD])          # Wk_rope.T * s2, [p,kt,m]
    wqf = inp("wqf", [P, 6 * 2 * 768], f8)  # [Wq_up|Wq_rope].T * s2*64
    wku = inp("wku", [P, 4 * 512])          # Wk_up_g.T * s2, [p,kt,m]
    wvu = inp("wvu", [P, 4 * 512])          # Wv_up_g.T
    wo = inp("wo", [P, 4 * D])              # Wo[:, cols_g].T, [p,kt,m]
    cosb = inp("cosb", [P, S])              # bf16 rope tables, 4x partition-tiled
    sinb = inp("sinb", [P, S])
    masks = inp("masks", [P, G * PAN])      # multiplicative causal masks
    onc = inp("onc", [P, 1])                # ones column
    y = nc.dram_tensor("y", [S, D], bf16, kind="ExternalOutput").ap()

    KT_D = D // P      # 16 k-tiles over model dim
    KT_KV = KVR // P   # 4
    NP = S // PAN      # 4 panels
    GROUPS = [[0, 1, 2, 3], [4, 5, 6, 7]]
    INV64 = 1.0 / 64.0

    with tile.TileContext(nc) as tc:
        with (
            tc.tile_pool(name="res", bufs=1) as res,
            tc.tile_pool(name="panels", bufs=8) as panels,
            tc.tile_pool(name="work", bufs=2) as work,
            tc.tile_pool(name="dram", bufs=1, space="DRAM") as dram,
        ):
            # ---- SBUF residents ----------------------------------------
            # q/k fp8 layout [P, head, slot, S]: slot 0 = nope dims,
            # slot 1 rows 0:64 = roped rope dims, rows 64:128 zero.
            q_sb = res.tile([P, HL, 2, S], f8, tag="q")
            k_sb = res.tile([P, HL, 2, S], f8, tag="k")
            v_sb = res.tile([P, S // P, 512], bf16, tag="v")
            ao_sb = res.tile([P, HL, S], bf16, tag="ao")
            masks_sb = res.tile([P, G, PAN], bf16, tag="masks")
            onc_sb = res.tile([P, 1], bf16, tag="onc")
            cos_sb = res.tile([P, S], bf16, tag="cos")
            sin_sb = res.tile([P, S], bf16, tag="sin")

            # zero the unused upper rope rows so the fp8 matmul never sees
            # garbage (e4m3 NaN patterns would poison 0*NaN)
            nc.vector.memset(q_sb[64:128, :, 1, :], 0.0)
            nc.vector.memset(k_sb[64:128, :, 1, :], 0.0)
            nc.gpsimd.dma_start(
                masks_sb[:], masks.rearrange("p (j q) -> p j q", q=PAN)
            )
            nc.gpsimd.dma_start(onc_sb[:], onc[:])
            nc.gpsimd.dma_start(cos_sb[:], cosb[:])
            nc.gpsimd.dma_start(sin_sb[:], sinb[:])

            # ---- DRAM staging for the latent AllGather ------------------
            ag_in = [dram.tile([GR, PAN], bf16, tag=f"agi{n}", name=f"agi{n}")
                     for n in range(NP)]
            ag_out = [dram.tile([G * GR, PAN], bf16, tag=f"ago{n}",
                                name=f"ago{n}") for n in range(NP)]

            def f8rows(t):
                # [rows, PAN] bf16 -> [2*rows, PAN] fp8 view of same bytes
                return t[:].bitcast(f8).rearrange("r (two s) -> (r two) s",
                                                  s=PAN)

            def rope_block(dst64, src64, ns, b0):
                # dst [64, PAN] fp8 slot-1 rows; src [64, PAN] bf16 whose
                # halves sit at absolute partitions b0 and b0+32 (the BIR
                # verifier wants matching base partitions for SBUF inputs,
                # hence the partition-tiled cos/sin tables)
                cs0, cs1 = cos_sb[b0 : b0 + 32, ns], cos_sb[b0 + 32 : b0 + 64, ns]
                sn0, sn1 = sin_sb[b0 : b0 + 32, ns], sin_sb[b0 + 32 : b0 + 64, ns]
                t1 = work.tile([32, PAN], bf16, tag="rt1")
                t2 = work.tile([32, PAN], bf16, tag="rt2")
                nc.vector.tensor_mul(t1, src64[0:32, :], cs0)
                nc.vector.tensor_mul(t2, src64[32:64, :], sn1)
                nc.vector.tensor_sub(dst64[0:32, :], t1, t2)
                t3 = work.tile([32, PAN], bf16, tag="rt1")
                t4 = work.tile([32, PAN], bf16, tag="rt2")
                nc.vector.tensor_mul(t3, src64[32:64, :], cs1)
                nc.vector.tensor_mul(t4, src64[0:32, :], sn0)
                nc.vector.tensor_add(dst64[32:64, :], t3, t4)

            # ---- Phase A + B -------------------------------------------
            with (
                tc.tile_pool(name="pa", bufs=1) as pa,
                tc.tile_pool(name="pb", bufs=1) as pb,
                tc.tile_pool(name="bq", bufs=3) as bq,
                tc.tile_pool(name="psA", bufs=4, space="PSUM") as psA,
                tc.tile_pool(name="psB", bufs=3, space="PSUM") as psB,
            ):
                # A weights: wqd8 chunked so the first matmul starts early
                wqd_sb = pa.tile([P, KT_D // 2, 2, QSH], f8, tag="wqd8")
                x80_ch = []
                for c in range(4):
                    nc.sync.dma_start(wqd_sb[:, 2 * c : 2 * c + 2, :, :],
                                      wqd8[:, c * 2 * 2 * QSH : (c + 1) * 2 * 2 * QSH])
                    t = panels.tile([P, 2, 2, PAN], f8, tag="panel8",
                                    name=f"x8_sb0_{c}")
                    nc.sync.dma_start(t[:], xp8[:, c * 4 * PAN : (c + 1) * 4 * PAN])
                    x80_ch.append(t)
                wkvd_sb = pa.tile([P, KT_D, KSH], bf16, tag="wkvd")
                nc.sync.dma_start(wkvd_sb[:], wkvd[:])
                wkr_sb = pa.tile([P, KT_D, RD], bf16, tag="wkr")
                nc.sync.dma_start(wkr_sb[:], wkr[:])
                x0_ch = []
                for c in range(4):
                    t = panels.tile([P, 4, PAN], bf16, tag="panel",
                                    name=f"x_sb0_{c}")
                    (nc.sync if c % 2 == 0 else nc.scalar).dma_start(
                        t[:], xp[:, c * 4 * PAN : (c + 1) * 4 * PAN])
                    x0_ch.append(t)

                # B weights on the gpsimd DMA queue, off the hot SP queue
                def load_b_weights():
                    wqf_sb = pb.tile([P, 6, 2, 768], f8, tag="wqf")
                    nc.gpsimd.dma_start(wqf_sb[:], wqf[:])
                    wku_sb = pb.tile([P, KT_KV, 512], bf16, tag="wku")
                    nc.gpsimd.dma_start(wku_sb[:], wku[:])
                    wvu_sb = pb.tile([P, KT_KV, 512], bf16, tag="wvu")
                    nc.gpsimd.dma_start(wvu_sb[:], wvu[:])
                    return wqf_sb, wku_sb, wvu_sb

                def phase_a(n):
                    """my latent slices for panel n + local k_rope, then gather"""
                    ns = slice(n * PAN, (n + 1) * PAN)
                    if n == 0:
                        x_ch, x8_ch = x0_ch, x80_ch
                    else:
                        x8_ch = []
                        for c in range(4):
                            t = panels.tile([P, 2, 2, PAN], f8, tag="panel8",
                                            name=f"x8_sb{n}_{c}")
                            o = (n * 16 + c * 4) * PAN
                            nc.sync.dma_start(t[:], xp8[:, o : o + 4 * PAN])
                            x8_ch.append(t)
                        x_ch = []
                        for c in range(4):
                            t = panels.tile([P, 4, PAN], bf16, tag="panel",
                                            name=f"x_sb{n}_{c}")
                            o = (n * 16 + c * 4) * PAN
                            (nc.sync if c % 2 == 0 else nc.scalar).dma_start(
                                t[:], xp[:, o : o + 4 * PAN])
                            x_ch.append(t)
                    gin8 = f8rows(ag_in[n])
                    # c_q in fp8 DoubleRow: kt-pair-major over 3 concurrent
                    # psums so matmuls start as soon as the first chunk lands
                    pss = [psA.tile([P, PAN], f32, tag="psA", name=f"psA{n}_{m}")
                           for m in range(4)]
                    for j in range(KT_D // 2):
                        xk = x8_ch[j // 2][:, j % 2, :, :]
                        st0, st1 = (j == 0), (j == KT_D // 2 - 1)
                        for m in range(3):
                            nc.tensor.matmul(
                                pss[m],
                                lhsT=wqd_sb[:, j, :, m * P : (m + 1) * P],
                                rhs=xk, start=st0, stop=st1, perf_mode=DR,
                            )
                    # c_kv (bf16, feeds v) + local k_rope share the x chunks
                    ps_kr = psA.tile([64, PAN], f32, tag="psKR", bufs=1)
                    for kt in range(KT_D):
                        xk = x_ch[kt // 4][:, kt % 4, :]
                        st0, st1 = (kt == 0), (kt == KT_D - 1)
                        nc.tensor.matmul(pss[3], lhsT=wkvd_sb[:, kt, :],
                                         rhs=xk, start=st0, stop=st1)
                        nc.tensor.matmul(ps_kr, lhsT=wkr_sb[:, kt, :],
                                         rhs=xk, start=st0, stop=st1)
                    cqr = gin8[0:384, :].rearrange("(p r) s -> p r s", r=3)
                    for m in range(3):  # c_q -> fp8 staging (scale 1/64 back)
                        st = work.tile([P, PAN], f8, tag="cq8")
                        nc.scalar.activation(st, pss[m], AF.Copy, scale=INV64)
                        nc.sync.dma_start(cqr[:, m, :], st)
                    st = work.tile([P, PAN], bf16, tag="ckvst")
                    nc.scalar.activation(st, pss[3], AF.Copy)
                    nc.sync.dma_start(ag_in[n][192:320, :], st)
                    nc.gpsimd.collective_compute(
                        "AllGather", mybir.AluOpType.bypass,
                        replica_groups=GROUPS,
                        ins=[ag_in[n].opt()], outs=[ag_out[n].opt()],
                    )
                    krb = work.tile([64, PAN], bf16, tag="krb")
                    nc.scalar.activation(krb, ps_kr, AF.Copy)
                    rope_block(k_sb[0:64, 0, 1, ns], krb, ns, 0)
                    for hh in range(1, HL):  # replicate to the other heads
                        nc.vector.tensor_copy(k_sb[0:64, hh, 1, ns],
                                              k_sb[0:64, 0, 1, ns])

                def phase_b(n):
                    """up-projections for panel n from the gathered latents"""
                    ns = slice(n * PAN, (n + 1) * PAN)
                    gout8 = f8rows(ag_out[n])
                    cqf = bq.tile([P, 12, PAN], f8, tag="cqf")
                    nc.scalar.dma_start(
                        cqf[:],
                        gout8.rearrange("(gi x) s -> gi x s", x=2 * GR)[
                            :, 0:QSH, :
                        ].rearrange("gi (p r) s -> p gi r s", r=3),
                    )
                    ckv = bq.tile([P, KT_KV, PAN], bf16, tag="ckv")
                    nc.scalar.dma_start(
                        ckv[:],
                        ag_out[n].rearrange("(gi x) s -> gi x s", x=GR)[
                            :, 192:320, :
                        ].rearrange("gi p s -> p gi s"),
                    )
                    for m in range(HL):  # q nope heads (fp8 DoubleRow)
                        ps = psB.tile([P, PAN], f32, tag="psB")
                        for j in range(6):
                            nc.tensor.matmul(
                                ps,
                                lhsT=wqf_sb[:, j, :, m * P : (m + 1) * P],
                                rhs=cqf[:, 2 * j : 2 * j + 2, :],
                                start=(j == 0), stop=(j == 5), perf_mode=DR,
                            )
                        nc.scalar.activation(q_sb[:, m, 0, ns], ps, AF.Copy,
                                             scale=INV64)
                    for hp in range(HL // 2):  # rope: two heads per psum
                        c0 = 512 + 128 * hp
                        ps = psB.tile([P, PAN], f32, tag="psB")
                        for j in range(6):
                            nc.tensor.matmul(
                                ps,
                                lhsT=wqf_sb[:, j, :, c0 : c0 + 128],
                                rhs=cqf[:, 2 * j : 2 * j + 2, :],
                                start=(j == 0), stop=(j == 5), perf_mode=DR,
                            )
                        qrp = work.tile([P, PAN], bf16, tag="qrp")
                        nc.scalar.activation(qrp, ps, AF.Copy, scale=INV64)
                        rope_block(q_sb[0:64, 2 * hp, 1, ns], qrp[0:64, :], ns, 0)
                        rope_block(q_sb[0:64, 2 * hp + 1, 1, ns],
                                   qrp[64:128, :], ns, 64)
                    for m in range(HL):  # k_c
                        ps = psB.tile([P, PAN], f32, tag="psB")
                        for kt in range(KT_KV):
                            nc.tensor.matmul(
                                ps,
                                lhsT=wku_sb[:, kt, m * P : (m + 1) * P],
                                rhs=ckv[:, kt, :],
                                start=(kt == 0), stop=(kt == KT_KV - 1),
                            )
                        nc.scalar.activation(k_sb[:, m, 0, ns], ps, AF.Copy)
                    for sti in range(4):  # v for this panel's S-tiles
                        ps = psB.tile([P, PAN], f32, tag="psB")
                        for kt in range(KT_KV):
                            nc.tensor.matmul(
                                ps,
                                lhsT=ckv[:, kt, sti * P : (sti + 1) * P],
                                rhs=wvu_sb[:, kt, :],
                                start=(kt == 0), stop=(kt == KT_KV - 1),
                            )
                        nc.scalar.activation(v_sb[:, 4 * n + sti, :], ps, AF.Copy)

                phase_a(0)
                wqf_sb, wku_sb, wvu_sb = load_b_weights()
                phase_a(1)
                phase_a(2)
                phase_b(0)
                phase_a(3)
                phase_b(1)
                phase_b(2)
                phase_b(3)

            # ---------------- Phase C: SDPA + Phase D interleaved --------
            if "C" not in phases:
                # timing-partial build: consume B outputs so nothing is elided
                nc.gpsimd.dma_start(y[0:P, 0:PAN], q_sb[:, 0, 0, 0:PAN])
                nc.gpsimd.dma_start(y[P : 2 * P, 0:PAN], k_sb[:, 0, 0, 0:PAN])
                nc.gpsimd.dma_start(y[2 * P : 3 * P, 0:PAN], v_sb[:, 0, 0:PAN])
            elif True:
                do_cd(tc, nc, phases, q_sb, k_sb, v_sb, ao_sb, masks_sb,
                      onc_sb, wo, work, y)

    nc.compile()
    return nc


def do_cd(tc, nc, phases, q_sb, k_sb, v_sb, ao_sb, masks_sb, onc_sb, wo,
          work, y):
    import concourse.mybir as mybir

    dt = mybir.dt
    f32, bf16 = dt.float32, dt.bfloat16
    AF = mybir.ActivationFunctionType
    DR = mybir.MatmulPerfMode.DoubleRow
    if True:
        if True:
            with (
                tc.tile_pool(name="pw", bufs=1) as pw,
                tc.tile_pool(name="pe", bufs=10) as pe,
                tc.tile_pool(name="pds", bufs=2) as pds,
                tc.tile_pool(name="py", bufs=4) as py,
                tc.tile_pool(name="psS", bufs=2, space="PSUM") as psS,
                tc.tile_pool(name="psO", bufs=2, space="PSUM") as psO,
                tc.tile_pool(name="psDn", bufs=1, space="PSUM") as psDn,
                tc.tile_pool(name="psD", bufs=3, space="PSUM") as psD,
            ):
                wo_sb = pw.tile([P, HL, D], bf16, tag="wo")
                nc.gpsimd.dma_start(wo_sb[:], wo[:])
                ycnt = [0]

                def d_unit(m, nh, tail=False):
                    """one output-projection unit: 2 nn-blocks of row-tile m"""
                    ms = slice(m * P, (m + 1) * P)
                    yst = py.tile([P, 2 * PAN], bf16, tag="yst")
                    for sub in range(2):
                        nn = 2 * nh + sub
                        ps = psD.tile([P, PAN], f32, tag="psD")
                        for kt in range(HL):
                            nc.tensor.matmul(
                                ps,
                                lhsT=ao_sb[:, kt, ms],
                                rhs=wo_sb[:, kt, nn * PAN : (nn + 1) * PAN],
                                start=(kt == 0), stop=(kt == HL - 1),
                            )
                        nc.vector.tensor_copy(
                            yst[:, sub * PAN : (sub + 1) * PAN], ps)
                    q = (nc.sync, nc.gpsimd, nc.scalar)[ycnt[0] % 3]
                    ycnt[0] += 1
                    q.dma_start(y[ms, 2 * nh * PAN : (2 * nh + 2) * PAN], yst)

                dq = []  # pending D units, drained inside the SDPA stream

                def d_fill():
                    if dq and "D" in phases:
                        m, nh = dq.pop(0)
                        d_unit(m, nh)

                def sdpa_pair(g, h0):
                    """two heads' SDPA chains interleaved kb-by-kb so the
                    in-order PE queue always has independent ready work while
                    the other chain waits on its exp"""
                    gs = slice(g * PAN, (g + 1) * PAN)
                    nk = 4 * (g + 1)
                    hs = (h0, h0 + 1)
                    ps_o = {h: psO.tile([P, PAN], f32, tag="ps_o",
                                        name=f"pso{g}_{h}") for h in hs}
                    # both chains' denominators live in one psum bank, at
                    # partition bases 0 and 32 (legal matmul tile positions)
                    ps_d2 = psDn.tile([33, PAN], f32, tag="ps_d")
                    e_t = {h: [None] * nk for h in hs}
                    dsum = {h: [None] * (nk // 2) for h in hs}
                    qsum = {h: [None] * (nk // 4) for h in hs}
                    pend = {h: [] for h in hs}

                    def consume(ci, h, kb):
                        nc.tensor.matmul(
                            ps_o[h], lhsT=v_sb[:, kb, h * P : (h + 1) * P],
                            rhs=e_t[h][kb],
                            start=(kb == 0), stop=(kb == nk - 1),
                        )
                        if kb % 4 == 3:
                            nc.tensor.matmul(
                                ps_d2[32 * ci : 32 * ci + 1, :],
                                lhsT=onc_sb[:], rhs=qsum[h][kb // 4],
                                start=(kb == 3), stop=(kb == nk - 1),
                                skip_group_check=True,
                            )

                    for kb in range(nk):
                        ks = slice(kb * P, (kb + 1) * P)
                        for ci, h in enumerate(hs):
                            ps_s = psS.tile([P, PAN], f32, tag="ps_s")
                            nc.tensor.matmul(
                                ps_s, lhsT=k_sb[:, h, :, ks],
                                rhs=q_sb[:, h, :, gs],
                                start=True, stop=True, perf_mode=DR,
                            )
                            e = pe.tile([P, PAN], bf16, tag="e")
                            nc.scalar.activation(e, ps_s, AF.Exp)
                            if kb >= 4 * g:
                                nc.vector.tensor_mul(
                                    e, e, masks_sb[:, kb - 4 * g, :])
                            e_t[h][kb] = e
                            if kb % 2 == 1:
                                ds = pds.tile([P, PAN], bf16, tag=f"dsum{ci}")
                                nc.vector.tensor_add(ds, e_t[h][kb - 1],
                                                     e_t[h][kb])
                                dsum[h][kb // 2] = ds
                            if kb % 4 == 3:
                                qs = pds.tile([P, PAN], bf16, tag=f"qsum{ci}")
                                nc.vector.tensor_add(qs, dsum[h][kb // 2 - 1],
                                                     dsum[h][kb // 2])
                                qsum[h][kb // 4] = qs
                            pend[h].append(kb)
                            if len(pend[h]) > 4:
                                consume(ci, h, pend[h].pop(0))
                    for ci, h in enumerate(hs):
                        while pend[h]:
                            consume(ci, h, pend[h].pop(0))
                    for ci, h in enumerate(hs):
                        rc = work.tile([1, PAN], f32, tag="rc")
                        nc.vector.reciprocal(rc, ps_d2[32 * ci : 32 * ci + 1, :])
                        bb = work.tile([P, PAN], f32, tag="bb")
                        nc.gpsimd.partition_broadcast(bb, rc)
                        nc.vector.tensor_mul(ao_sb[:, h, gs], ps_o[h], bb)

                for g in range(G):
                    if g >= 1:
                        dq.extend((4 * (g - 1) + mm, nh)
                                  for mm in range(4) for nh in range(2))
                    for hp in range(HL // 2):
                        sdpa_pair(g, 2 * hp)
                        for _ in range(4):
                            d_fill()
                    while dq:
                        d_fill()
                if "D" in phases:
                    for m in range(4 * (G - 1), 4 * G):
                        for nh in range(2):
                            d_unit(m, nh, tail=True)


def _prep_inputs(x, positions, Wq_down, Wq_up, Wq_rope, Wkv_down, Wk_up, Wv_up,
                 Wk_rope, Wo):
    s2 = np.float32(QK_D ** -0.25)  # sqrt of the 1/sqrt(qk_d) scale
    bf = lambda a: np.ascontiguousarray(a).astype(BF16)

    def pmaj(a, kt):
        # [kt*128, m] -> [128, kt*m] partition-major
        m = a.shape[1]
        return np.ascontiguousarray(
            a.reshape(kt, P, m).transpose(1, 0, 2).reshape(P, kt * m))

    inv_freq = 1.0 / (10000.0 ** (np.arange(0, RD, 2, dtype=np.float32) / RD))
    ang = positions.astype(np.float32)[:, None] * inv_freq  # (S, 32)
    shared = {
        "wkr": pmaj(Wk_rope.T * s2, 16).astype(BF16),
        "onc": np.ones((P, 1), BF16),
        "cosb": bf(np.tile(np.cos(ang).T, (4, 1))),
        "sinb": bf(np.tile(np.sin(ang).T, (4, 1))),
    }

    mk = np.zeros((P, G * PAN), np.float32)
    for j in range(G):
        p = np.arange(P)[:, None]
        q = np.arange(PAN)[None, :]
        mk[:, j * PAN : (j + 1) * PAN] = (j * P + p <= q).astype(np.float32)
    shared["masks"] = mk.astype(BF16)

    wqdT = Wq_down.T  # (D, QR)
    wkvdT = Wkv_down.T  # (D, KVR)
    per_g = []
    for g in range(G):
        rs, rr = slice(512 * g, 512 * (g + 1)), slice(256 * g, 256 * (g + 1))
        wqall = np.concatenate([Wq_up[rs].T, Wq_rope[rr].T], axis=1)
        per_g.append({
            "wqd8": pmaj(wqdT[:, QSH * g : QSH * (g + 1)] * 64.0, 16).astype(F8),
            "wkvd": pmaj(wkvdT[:, KSH * g : KSH * (g + 1)], 16).astype(BF16),
            "wqf": pmaj(wqall * (s2 * 64.0), 12).astype(F8),
            "wku": pmaj(Wk_up[rs].T * s2, 4).astype(BF16),
            "wvu": pmaj(Wv_up[rs].T, 4).astype(BF16),
            "wo": pmaj(Wo[:, rs].T, 4).astype(BF16),
        })
    # x as [p, panel, kt, s] partition-major flat
    def xprep(a):
        # a: [D, S] -> [128, NP*16*PAN]
        return np.ascontiguousarray(
            a.reshape(16, P, 4, PAN).transpose(1, 2, 0, 3).reshape(P, -1))

    xTs = [x[b].T for b in range(B)]
    xps = [xprep(a).astype(BF16) for a in xTs]
    xp8s = [xprep(a).astype(F8) for a in xTs]

    in_maps = []
    for c in range(8):
        b, g = c // G, c % G
        m = dict(shared)
        m.update(per_g[g])
        m["xp"] = xps[b]
        m["xp8"] = xp8s[b]
        in_maps.append(m)
    return in_maps


def kernel(**inputs):
    from concourse.bass_utils import run_bass_kernel_spmd

    if "nc" not in _cache:
        _cache["nc"] = _build_module()
    nc = _cache["nc"]

    in_maps = _prep_inputs(**inputs)
    res = None
    for attempt in range(3):
        try:
            res = run_bass_kernel_spmd(nc, in_maps, core_ids=list(range(8)))
            break
        except Exception:
            if attempt == 2:
                raise
    out = np.zeros((B, S, D), np.float32)
    for c in range(8):
        out[c // G] += res.results[c]["y"].astype(np.float32)
    return out


# revision 95
# speedup vs baseline: 1.0115x; 1.0059x over previous
"""Trainium2 Bass kernel for MultiLatentAttention (MLA).

Sharding: 8 cores = 2 (batch) x 4 (head-groups of 4 heads).
Within each batch group of 4 cores, the down-projections are sharded by
output rows and AllGathered (per S-panel, pipelined).  Each core computes
the shared k_rope head locally for all panels (no collective), runs its
4 heads' up-projections + SDPA and a partial output projection
y_part = attn_out @ Wo[:, heads].T.  Host sums the 4 partials per batch.

Speed tricks vs the bf16 baseline:
- Scores are computed with fp8e4 DoubleRow matmuls: the 192-dim qk
  contraction (128 nope + 64 rope) is packed as [128, 2, *] operands and
  runs at 0.5 cycles/column (4x fewer PE cycles than 2 bf16 matmuls).
- The q up-projection also runs in fp8 DoubleRow (k-tile pairs), with
  the fp8 weights pre-scaled by 64 to stay in e4m3 normal range; the 64
  is divided back out in the psum->q cast on the Act engine.
- c_q travels through the AllGather in fp8 (mixed fp8/bf16 gather buffer
  via bitcast views), shrinking collective bytes and killing the
  readback cast.
- Softmax denominators use DVE pair-sums of e-tiles, halving the
  ones-matmul count; attention output stays in SBUF (no DRAM roundtrip)
  and the output projection is software-pipelined into the SDPA stream.
- 1/sqrt(192) is split as s2 = 192**-0.25 folded into both the q-side
  and k-side weights so fp8 operands are magnitude-balanced.
All fp8/bf16 matmuls accumulate in f32 PSUM; y partials are f32.
"""

import sys

if "/opt/trn_rl_repo" not in sys.path:
    sys.path.insert(0, "/opt/trn_rl_repo")

import numpy as np
import ml_dtypes

BF16 = ml_dtypes.bfloat16
F8 = ml_dtypes.float8_e4m3

B, S, D, H = 2, 2048, 2048, 16
QR, KVR = 1536, 512
NOPE, RD, VD = 128, 64, 128
QK_D = NOPE + RD
HL = 4          # heads per core
G = 4           # head groups (= cores per batch group)
QSH = QR // G   # 384 c_q rows per core
KSH = KVR // G  # 128 c_kv rows per core
PAN = 512       # panel width
P = 128
GR = 320        # gather rows (bf16 units): 192 (cq fp8) + 128 (ckv bf16)

_cache = {}


def _build_module(phases="ABCD"):
    import concourse.bacc as bacc
    import concourse.mybir as mybir
    import concourse.tile as tile

    dt = mybir.dt
    f32, bf16, f8 = dt.float32, dt.bfloat16, dt.float8e4
    AF = mybir.ActivationFunctionType
    DR = mybir.MatmulPerfMode.DoubleRow

    nc = bacc.Bacc("TRN2", target_bir_lowering=False, debug=False, num_devices=8)

    def inp(name, shape, dtype=bf16):
        return nc.dram_tensor(name, shape, dtype, kind="ExternalInput").ap()

    # all inputs arrive pre-arranged partition-major ([128, flat]) so every
    # load is a contiguous 2-D DMA (128 descriptors, no strided gather)
    xp = inp("xp", [P, 4 * 16 * PAN])       # x[b].T as [p, panel, kt, s]
    xp8 = inp("xp8", [P, 4 * 16 * PAN], f8)
    wqd8 = inp("wqd8", [P, 8 * 2 * QSH], f8)   # Wq_down.T slice * 64, [p,j,two,m]
    wkvd = inp("wkvd", [P, 16 * KSH])       # Wkv_down.T slice, [p,kt,m]
    wkr = inp("wkr", [P, 16 * RD])          # Wk_rope.T * s2, [p,kt,m]
    wqf = inp("wqf", [P, 6 * 2 * 768], f8)  # [Wq_up|Wq_rope].T * s2*64
    wku = inp("wku", [P, 4 * 512])          # Wk_up_g.T * s2, [p,kt,m]
    wvu = inp("wvu", [P, 4 * 512])          # Wv_up_g.T
    wo = inp("wo", [P, 4 * D])              # Wo[:, cols_g].T, [p,kt,m]
    cosb = inp("cosb", [P, S])              # bf16 rope tables, 4x partition-tiled
    sinb = inp("sinb", [P, S])
    masks = inp("masks", [P, G * PAN])      # multiplicative causal masks
    onc = inp("onc", [P, 1])                # ones column
    y = nc.dram_tensor("y", [S, D], bf16, kind="ExternalOutput").ap()

    KT_D = D // P      # 16 k-tiles over model dim
    KT_KV = KVR // P   # 4
    NP = S // PAN      # 4 panels
    GROUPS = [[0, 1, 2, 3], [4, 5, 6, 7]]
    INV64 = 1.0 / 64.0

    with tile.TileContext(nc) as tc:
        with (
            tc.tile_pool(name="res", bufs=1) as res,
            tc.tile_pool(name="panels", bufs=8) as panels,
            tc.tile_pool(name="work", bufs=2) as work,
            tc.tile_pool(name="dram", bufs=1, space="DRAM") as dram,
        ):
            # ---- SBUF residents ----------------------------------------
            # q/k fp8 layout [P, head, slot, S]: slot 0 = nope dims,
            # slot 1 rows 0:64 = roped rope dims, rows 64:128 zero.
            q_sb = res.tile([P, HL, 2, S], f8, tag="q")
            k_sb = res.tile([P, HL, 2, S], f8, tag="k")
            v_sb = res.tile([P, S // P, 512], bf16, tag="v")
            ao_sb = res.tile([P, HL, S], bf16, tag="ao")
            masks_sb = res.tile([P, G, PAN], bf16, tag="masks")
            onc_sb = res.tile([P, 1], bf16, tag="onc")
            cos_sb = res.tile([P, S], bf16, tag="cos")
            sin_sb = res.tile([P, S], bf16, tag="sin")

            # zero the unused upper rope rows so the fp8 matmul never sees
            # garbage (e4m3 NaN patterns would poison 0*NaN)
            nc.vector.memset(q_sb[64:128, :, 1, :], 0.0)
            nc.vector.memset(k_sb[64:128, :, 1, :], 0.0)
            nc.gpsimd.dma_start(
                masks_sb[:], masks.rearrange("p (j q) -> p j q", q=PAN)
            )
            nc.gpsimd.dma_start(onc_sb[:], onc[:])
            nc.gpsimd.dma_start(cos_sb[:], cosb[:])
            nc.gpsimd.dma_start(sin_sb[:], sinb[:])

            # ---- DRAM staging for the latent AllGather ------------------
            ag_in = [dram.tile([GR, PAN], bf16, tag=f"agi{n}", name=f"agi{n}")
                     for n in range(NP)]
            ag_out = [dram.tile([G * GR, PAN], bf16, tag=f"ago{n}",
                                name=f"ago{n}") for n in range(NP)]

            def f8rows(t):
                # [rows, PAN] bf16 -> [2*rows, PAN] fp8 view of same bytes
                return t[:].bitcast(f8).rearrange("r (two s) -> (r two) s",
                                                  s=PAN)

            def rope_block(dst64, src64, ns, b0):
                # dst [64, PAN] fp8 slot-1 rows; src [64, PAN] bf16 whose
                # halves sit at absolute partitions b0 and b0+32 (the BIR
                # verifier wants matching base partitions for SBUF inputs,
                # hence the partition-tiled cos/sin tables)
                cs0, cs1 = cos_sb[b0 : b0 + 32, ns], cos_sb[b0 + 32 : b0 + 64, ns]
                sn0, sn1 = sin_sb[b0 : b0 + 32, ns], sin_sb[b0 + 32 : b0 + 64, ns]
                t1 = work.tile([32, PAN], bf16, tag="rt1")
                t2 = work.tile([32, PAN], bf16, tag="rt2")
                nc.vector.tensor_mul(t1, src64[0:32, :], cs0)
                nc.vector.tensor_mul(t2, src64[32:64, :], sn1)
                nc.vector.tensor_sub(dst64[0:32, :], t1, t2)
                t3 = work.tile([32, PAN], bf16, tag="rt1")
                t4 = work.tile([32, PAN], bf16, tag="rt2")
                nc.vector.tensor_mul(t3, src64[32:64, :], cs1)
                nc.vector.tensor_mul(t4, src64[0:32, :], sn0)
                nc.vector.tensor_add(dst64[32:64, :], t3, t4)

            # ---- Phase A + B -------------------------------------------
            with (
                tc.tile_pool(name="pa", bufs=1) as pa,
                tc.tile_pool(name="pb", bufs=1) as pb,
                tc.tile_pool(name="bq", bufs=3) as bq,
                tc.tile_pool(name="psA", bufs=4, space="PSUM") as psA,
                tc.tile_pool(name="psB", bufs=3, space="PSUM") as psB,
            ):
                # A weights: wqd8 chunked so the first matmul starts early
                wqd_sb = pa.tile([P, KT_D // 2, 2, QSH], f8, tag="wqd8")
                x80_ch = []
                for c in range(4):
                    nc.sync.dma_start(wqd_sb[:, 2 * c : 2 * c + 2, :, :],
                                      wqd8[:, c * 2 * 2 * QSH : (c + 1) * 2 * 2 * QSH])
                    t = panels.tile([P, 2, 2, PAN], f8, tag="panel8",
                                    name=f"x8_sb0_{c}")
                    nc.sync.dma_start(t[:], xp8[:, c * 4 * PAN : (c + 1) * 4 * PAN])
                    x80_ch.append(t)
                wkvd_sb = pa.tile([P, KT_D, KSH], bf16, tag="wkvd")
                nc.sync.dma_start(wkvd_sb[:], wkvd[:])
                wkr_sb = pa.tile([P, KT_D, RD], bf16, tag="wkr")
                nc.sync.dma_start(wkr_sb[:], wkr[:])
                x0_ch = []
                for c in range(4):
                    t = panels.tile([P, 4, PAN], bf16, tag="panel",
                                    name=f"x_sb0_{c}")
                    (nc.sync if c % 2 == 0 else nc.scalar).dma_start(
                        t[:], xp[:, c * 4 * PAN : (c + 1) * 4 * PAN])
                    x0_ch.append(t)

                # B weights on the gpsimd DMA queue, off the hot SP queue
                def load_b_weights():
                    wqf_sb = pb.tile([P, 6, 2, 768], f8, tag="wqf")
                    nc.gpsimd.dma_start(wqf_sb[:], wqf[:])
                    wku_sb = pb.tile([P, KT_KV, 512], bf16, tag="wku")
                    nc.gpsimd.dma_start(wku_sb[:], wku[:])
                    wvu_sb = pb.tile([P, KT_KV, 512], bf16, tag="wvu")
                    nc.gpsimd.dma_start(wvu_sb[:], wvu[:])
                    return wqf_sb, wku_sb, wvu_sb

                def phase_a(n):
                    """my latent slices for panel n + local k_rope, then gather"""
                    ns = slice(n * PAN, (n + 1) * PAN)
                    if n == 0:
                        x_ch, x8_ch = x0_ch, x80_ch
                    else:
                        x8_ch = []
                        for c in range(4):
                            t = panels.tile([P, 2, 2, PAN], f8, tag="panel8",
                                            name=f"x8_sb{n}_{c}")
                            o = (n * 16 + c * 4) * PAN
                            nc.sync.dma_start(t[:], xp8[:, o : o + 4 * PAN])
                            x8_ch.append(t)
                        x_ch = []
                        for c in range(4):
                            t = panels.tile([P, 4, PAN], bf16, tag="panel",
                                            name=f"x_sb{n}_{c}")
                            o = (n * 16 + c * 4) * PAN
                            (nc.sync if c % 2 == 0 else nc.scalar).dma_start(
                                t[:], xp[:, o : o + 4 * PAN])
                            x_ch.append(t)
                    gin8 = f8rows(ag_in[n])
                    # c_q in fp8 DoubleRow: kt-pair-major over 3 concurrent
                    # psums so matmuls start as soon as the first chunk lands
                    pss = [psA.tile([P, PAN], f32, tag="psA", name=f"psA{n}_{m}")
                           for m in range(4)]
                    for j in range(KT_D // 2):
                        xk = x8_ch[j // 2][:, j % 2, :, :]
                        st0, st1 = (j == 0), (j == KT_D // 2 - 1)
                        for m in range(3):
                            nc.tensor.matmul(
                                pss[m],
                                lhsT=wqd_sb[:, j, :, m * P : (m + 1) * P],
                                rhs=xk, start=st0, stop=st1, perf_mode=DR,
                            )
                    # c_kv (bf16, feeds v) + local k_rope share the x chunks
                    ps_kr = psA.tile([64, PAN], f32, tag="psKR", bufs=1)
                    for kt in range(KT_D):
                        xk = x_ch[kt // 4][:, kt % 4, :]
                        st0, st1 = (kt == 0), (kt == KT_D - 1)
                        nc.tensor.matmul(pss[3], lhsT=wkvd_sb[:, kt, :],
                                         rhs=xk, start=st0, stop=st1)
                        nc.tensor.matmul(ps_kr, lhsT=wkr_sb[:, kt, :],
                                         rhs=xk, start=st0, stop=st1)
                    cqr = gin8[0:384, :].rearrange("(p r) s -> p r s", r=3)
                    for m in range(3):  # c_q -> fp8 staging (scale 1/64 back)
                        st = work.tile([P, PAN], f8, tag="cq8")
                        nc.scalar.activation(st, pss[m], AF.Copy, scale=INV64)
                        nc.sync.dma_start(cqr[:, m, :], st)
                    st = work.tile([P, PAN], bf16, tag="ckvst")
                    nc.scalar.activation(st, pss[3], AF.Copy)
                    nc.sync.dma_start(ag_in[n][192:320, :], st)
                    nc.gpsimd.collective_compute(
                        "AllGather", mybir.AluOpType.bypass,
                        replica_groups=GROUPS,
                        ins=[ag_in[n].opt()], outs=[ag_out[n].opt()],
                    )
                    krb = work.tile([64, PAN], bf16, tag="krb")
                    nc.scalar.activation(krb, ps_kr, AF.Copy)
                    rope_block(k_sb[0:64, 0, 1, ns], krb, ns, 0)
                    for hh in range(1, HL):  # replicate to the other heads
                        nc.vector.tensor_copy(k_sb[0:64, hh, 1, ns],
                                              k_sb[0:64, 0, 1, ns])

                def phase_b(n):
                    """up-projections for panel n from the gathered latents"""
                    ns = slice(n * PAN, (n + 1) * PAN)
                    gout8 = f8rows(ag_out[n])
                    cqf = bq.tile([P, 12, PAN], f8, tag="cqf")
                    nc.scalar.dma_start(
                        cqf[:],
                        gout8.rearrange("(gi x) s -> gi x s", x=2 * GR)[
                            :, 0:QSH, :
                        ].rearrange("gi (p r) s -> p gi r s", r=3),
                    )
                    ckv = bq.tile([P, KT_KV, PAN], bf16, tag="ckv")
                    nc.scalar.dma_start(
                        ckv[:],
                        ag_out[n].rearrange("(gi x) s -> gi x s", x=GR)[
                            :, 192:320, :
                        ].rearrange("gi p s -> p gi s"),
                    )
                    for m in range(HL):  # q nope heads (fp8 DoubleRow)
                        ps = psB.tile([P, PAN], f32, tag="psB")
                        for j in range(6):
                            nc.tensor.matmul(
                                ps,
                                lhsT=wqf_sb[:, j, :, m * P : (m + 1) * P],
                                rhs=cqf[:, 2 * j : 2 * j + 2, :],
                                start=(j == 0), stop=(j == 5), perf_mode=DR,
                            )
                        nc.scalar.activation(q_sb[:, m, 0, ns], ps, AF.Copy,
                                             scale=INV64)
                    for hp in range(HL // 2):  # rope: two heads per psum
                        c0 = 512 + 128 * hp
                        ps = psB.tile([P, PAN], f32, tag="psB")
                        for j in range(6):
                            nc.tensor.matmul(
                                ps,
                                lhsT=wqf_sb[:, j, :, c0 : c0 + 128],
                                rhs=cqf[:, 2 * j : 2 * j + 2, :],
                                start=(j == 0), stop=(j == 5), perf_mode=DR,
                            )
                        qrp = work.tile([P, PAN], bf16, tag="qrp")
                        nc.scalar.activation(qrp, ps, AF.Copy, scale=INV64)
                        rope_block(q_sb[0:64, 2 * hp, 1, ns], qrp[0:64, :], ns, 0)
                        rope_block(q_sb[0:64, 2 * hp + 1, 1, ns],
                                   qrp[64:128, :], ns, 64)
                    for m in range(HL):  # k_c
                        ps = psB.tile([P, PAN], f32, tag="psB")
                        for kt in range(KT_KV):
                            nc.tensor.matmul(
                                ps,
                                lhsT=wku_sb[:, kt, m * P : (m + 1) * P],
                                rhs=ckv[:, kt, :],
                                start=(kt == 0), stop=(kt == KT_KV - 1),
                            )
                        nc.scalar.activation(k_sb[:, m, 0, ns], ps, AF.Copy)
                    for sti in range(4):  # v for this panel's S-tiles
                        ps = psB.tile([P, PAN], f32, tag="psB")
                        for kt in range(KT_KV):
                            nc.tensor.matmul(
                                ps,
                                lhsT=ckv[:, kt, sti * P : (sti + 1) * P],
                                rhs=wvu_sb[:, kt, :],
                                start=(kt == 0), stop=(kt == KT_KV - 1),
                            )
                        nc.scalar.activation(v_sb[:, 4 * n + sti, :], ps, AF.Copy)

                phase_a(0)
                wqf_sb, wku_sb, wvu_sb = load_b_weights()
                phase_a(1)
                phase_a(2)
                phase_b(0)
                phase_a(3)
                phase_b(1)
                phase_b(2)
                phase_b(3)

            # ---------------- Phase C: SDPA + Phase D interleaved --------
            if "C" not in phases:
                # timing-partial build: consume B outputs so nothing is elided
                nc.gpsimd.dma_start(y[0:P, 0:PAN], q_sb[:, 0, 0, 0:PAN])
                nc.gpsimd.dma_start(y[P : 2 * P, 0:PAN], k_sb[:, 0, 0, 0:PAN])
                nc.gpsimd.dma_start(y[2 * P : 3 * P, 0:PAN], v_sb[:, 0, 0:PAN])
            elif True:
                do_cd(tc, nc, phases, q_sb, k_sb, v_sb, ao_sb, masks_sb,
                      onc_sb, wo, work, y)

    nc.compile()
    return nc


def do_cd(tc, nc, phases, q_sb, k_sb, v_sb, ao_sb, masks_sb, onc_sb, wo,
          work, y):
    import concourse.mybir as mybir

    dt = mybir.dt
    f32, bf16 = dt.float32, dt.bfloat16
    AF = mybir.ActivationFunctionType
    DR = mybir.MatmulPerfMode.DoubleRow
    if True:
        if True:
            with (
                tc.tile_pool(name="pw", bufs=1) as pw,
                tc.tile_pool(name="pe", bufs=18) as pe,
                tc.tile_pool(name="pds", bufs=2) as pds,
                tc.tile_pool(name="py", bufs=4) as py,
                tc.tile_pool(name="psS", bufs=2, space="PSUM") as psS,
                tc.tile_pool(name="psO", bufs=2, space="PSUM") as psO,
                tc.tile_pool(name="psDn", bufs=1, space="PSUM") as psDn,
                tc.tile_pool(name="psD", bufs=3, space="PSUM") as psD,
            ):
                wo_sb = pw.tile([P, HL, D], bf16, tag="wo")
                nc.gpsimd.dma_start(wo_sb[:], wo[:])
                ycnt = [0]

                def d_unit(m, nh, tail=False):
                    """one output-projection unit: 2 nn-blocks of row-tile m"""
                    ms = slice(m * P, (m + 1) * P)
                    yst = py.tile([P, 2 * PAN], bf16, tag="yst")
                    for sub in range(2):
                        nn = 2 * nh + sub
                        ps = psD.tile([P, PAN], f32, tag="psD")
                        for kt in range(HL):
                            nc.tensor.matmul(
                                ps,
                                lhsT=ao_sb[:, kt, ms],
                                rhs=wo_sb[:, kt, nn * PAN : (nn + 1) * PAN],
                                start=(kt == 0), stop=(kt == HL - 1),
                            )
                        nc.vector.tensor_copy(
                            yst[:, sub * PAN : (sub + 1) * PAN], ps)
                    q = (nc.sync, nc.gpsimd, nc.scalar)[ycnt[0] % 3]
                    ycnt[0] += 1
                    q.dma_start(y[ms, 2 * nh * PAN : (2 * nh + 2) * PAN], yst)

                dq = []  # pending D units, drained inside the SDPA stream

                def d_fill():
                    if dq and "D" in phases:
                        m, nh = dq.pop(0)
                        d_unit(m, nh)

                def sdpa_pair(g, h0):
                    """two heads' SDPA chains interleaved kb-by-kb so the
                    in-order PE queue always has independent ready work while
                    the other chain waits on its exp"""
                    gs = slice(g * PAN, (g + 1) * PAN)
                    nk = 4 * (g + 1)
                    hs = (h0, h0 + 1)
                    ps_o = {h: psO.tile([P, PAN], f32, tag="ps_o",
                                        name=f"pso{g}_{h}") for h in hs}
                    # both chains' denominators live in one psum bank, at
                    # partition bases 0 and 32 (legal matmul tile positions)
                    ps_d2 = psDn.tile([33, PAN], f32, tag="ps_d")
                    e_t = {h: [None] * nk for h in hs}
                    dsum = {h: [None] * (nk // 2) for h in hs}
                    qsum = {h: [None] * (nk // 4) for h in hs}
                    pend = {h: [] for h in hs}

                    def consume(ci, h, kb):
                        nc.tensor.matmul(
                            ps_o[h], lhsT=v_sb[:, kb, h * P : (h + 1) * P],
                            rhs=e_t[h][kb],
                            start=(kb == 0), stop=(kb == nk - 1),
                        )
                        if kb % 4 == 3:
                            nc.tensor.matmul(
                                ps_d2[32 * ci : 32 * ci + 1, :],
                                lhsT=onc_sb[:], rhs=qsum[h][kb // 4],
                                start=(kb == 3), stop=(kb == nk - 1),
                                skip_group_check=True,
                            )

                    for kb in range(nk):
                        ks = slice(kb * P, (kb + 1) * P)
                        for ci, h in enumerate(hs):
                            ps_s = psS.tile([P, PAN], f32, tag="ps_s")
                            nc.tensor.matmul(
                                ps_s, lhsT=k_sb[:, h, :, ks],
                                rhs=q_sb[:, h, :, gs],
                                start=True, stop=True, perf_mode=DR,
                            )
                            e = pe.tile([P, PAN], bf16, tag="e")
                            nc.scalar.activation(e, ps_s, AF.Exp)
                            if kb >= 4 * g:
                                nc.vector.tensor_mul(
                                    e, e, masks_sb[:, kb - 4 * g, :])
                            e_t[h][kb] = e
                            if kb % 2 == 1:
                                ds = pds.tile([P, PAN], bf16, tag=f"dsum{ci}")
                                nc.vector.tensor_add(ds, e_t[h][kb - 1],
                                                     e_t[h][kb])
                                dsum[h][kb // 2] = ds
                            if kb % 4 == 3:
                                qs = pds.tile([P, PAN], bf16, tag=f"qsum{ci}")
                                nc.vector.tensor_add(qs, dsum[h][kb // 2 - 1],
                                                     dsum[h][kb // 2])
                                qsum[h][kb // 4] = qs
                            pend[h].append(kb)
                            if len(pend[h]) > 8:
                                consume(ci, h, pend[h].pop(0))
                    for ci, h in enumerate(hs):
                        while pend[h]:
                            consume(ci, h, pend[h].pop(0))
                    for ci, h in enumerate(hs):
                        rc = work.tile([1, PAN], f32, tag="rc")
                        nc.vector.reciprocal(rc, ps_d2[32 * ci : 32 * ci + 1, :])
                        bb = work.tile([P, PAN], f32, tag="bb")
                        nc.gpsimd.partition_broadcast(bb, rc)
                        nc.vector.tensor_mul(ao_sb[:, h, gs], ps_o[h], bb)

                for g in range(G):
                    if g >= 1:
                        dq.extend((4 * (g - 1) + mm, nh)
                                  for mm in range(4) for nh in range(2))
                    for hp in range(HL // 2):
                        sdpa_pair(g, 2 * hp)
                        for _ in range(4):
                            d_fill()
                    while dq:
                        d_fill()
                if "D" in phases:
                    for m in range(4 * (G - 1), 4 * G):
                        for nh in range(2):
                            d_unit(m, nh, tail=True)


def _prep_inputs(x, positions, Wq_down, Wq_up, Wq_rope, Wkv_down, Wk_up, Wv_up,
                 Wk_rope, Wo):
    s2 = np.float32(QK_D ** -0.25)  # sqrt of the 1/sqrt(qk_d) scale
    bf = lambda a: np.ascontiguousarray(a).astype(BF16)

    def pmaj(a, kt):
        # [kt*128, m] -> [128, kt*m] partition-major
        m = a.shape[1]
        return np.ascontiguousarray(
            a.reshape(kt, P, m).transpose(1, 0, 2).reshape(P, kt * m))

    inv_freq = 1.0 / (10000.0 ** (np.arange(0, RD, 2, dtype=np.float32) / RD))
    ang = positions.astype(np.float32)[:, None] * inv_freq  # (S, 32)
    shared = {
        "wkr": pmaj(Wk_rope.T * s2, 16).astype(BF16),
        "onc": np.ones((P, 1), BF16),
        "cosb": bf(np.tile(np.cos(ang).T, (4, 1))),
        "sinb": bf(np.tile(np.sin(ang).T, (4, 1))),
    }

    mk = np.zeros((P, G * PAN), np.float32)
    for j in range(G):
        p = np.arange(P)[:, None]
        q = np.arange(PAN)[None, :]
        mk[:, j * PAN : (j + 1) * PAN] = (j * P + p <= q).astype(np.float32)
    shared["masks"] = mk.astype(BF16)

    wqdT = Wq_down.T  # (D, QR)
    wkvdT = Wkv_down.T  # (D, KVR)
    per_g = []
    for g in range(G):
        rs, rr = slice(512 * g, 512 * (g + 1)), slice(256 * g, 256 * (g + 1))
        wqall = np.concatenate([Wq_up[rs].T, Wq_rope[rr].T], axis=1)
        per_g.append({
            "wqd8": pmaj(wqdT[:, QSH * g : QSH * (g + 1)] * 64.0, 16).astype(F8),
            "wkvd": pmaj(wkvdT[:, KSH * g : KSH * (g + 1)], 16).astype(BF16),
            "wqf": pmaj(wqall * (s2 * 64.0), 12).astype(F8),
            "wku": pmaj(Wk_up[rs].T * s2, 4).astype(BF16),
            "wvu": pmaj(Wv_up[rs].T, 4).astype(BF16),
            "wo": pmaj(Wo[:, rs].T, 4).astype(BF16),
        })
    # x as [p, panel, kt, s] partition-major flat
    def xprep(a):
        # a: [D, S] -> [128, NP*16*PAN]
        return np.ascontiguousarray(
            a.reshape(16, P, 4, PAN).transpose(1, 2, 0, 3).reshape(P, -1))

    xTs = [x[b].T for b in range(B)]
    xps = [xprep(a).astype(BF16) for a in xTs]
    xp8s = [xprep(a).astype(F8) for a in xTs]

    in_maps = []
    for c in range(8):
        b, g = c // G, c % G
        m = dict(shared)
        m.update(per_g[g])
        m["xp"] = xps[b]
        m["xp8"] = xp8s[b]
        in_maps.append(m)
    return in_maps


def kernel(**inputs):
    from concourse.bass_utils import run_bass_kernel_spmd

    if "nc" not in _cache:
        _cache["nc"] = _build_module()
    nc = _cache["nc"]

    in_maps = _prep_inputs(**inputs)
    res = None
    for attempt in range(3):
        try:
            res = run_bass_kernel_spmd(nc, in_maps, core_ids=list(range(8)))
            break
        except Exception:
            if attempt == 2:
                raise
    out = np.zeros((B, S, D), np.float32)
    for c in range(8):
        out[c // G] += res.results[c]["y"].astype(np.float32)
    return out


# revision 96
# speedup vs baseline: 1.0116x; 1.0001x over previous
"""Trainium2 Bass kernel for MultiLatentAttention (MLA).

Sharding: 8 cores = 2 (batch) x 4 (head-groups of 4 heads).
Within each batch group of 4 cores, the down-projections are sharded by
output rows and AllGathered (per S-panel, pipelined).  Each core computes
the shared k_rope head locally for all panels (no collective), runs its
4 heads' up-projections + SDPA and a partial output projection
y_part = attn_out @ Wo[:, heads].T.  Host sums the 4 partials per batch.

Speed tricks vs the bf16 baseline:
- Scores are computed with fp8e4 DoubleRow matmuls: the 192-dim qk
  contraction (128 nope + 64 rope) is packed as [128, 2, *] operands and
  runs at 0.5 cycles/column (4x fewer PE cycles than 2 bf16 matmuls).
- The q up-projection also runs in fp8 DoubleRow (k-tile pairs), with
  the fp8 weights pre-scaled by 64 to stay in e4m3 normal range; the 64
  is divided back out in the psum->q cast on the Act engine.
- c_q travels through the AllGather in fp8 (mixed fp8/bf16 gather buffer
  via bitcast views), shrinking collective bytes and killing the
  readback cast.
- Softmax denominators use DVE pair-sums of e-tiles, halving the
  ones-matmul count; attention output stays in SBUF (no DRAM roundtrip)
  and the output projection is software-pipelined into the SDPA stream.
- 1/sqrt(192) is split as s2 = 192**-0.25 folded into both the q-side
  and k-side weights so fp8 operands are magnitude-balanced.
All fp8/bf16 matmuls accumulate in f32 PSUM; y partials are f32.
"""

import sys

if "/opt/trn_rl_repo" not in sys.path:
    sys.path.insert(0, "/opt/trn_rl_repo")

import numpy as np
import ml_dtypes

BF16 = ml_dtypes.bfloat16
F8 = ml_dtypes.float8_e4m3

B, S, D, H = 2, 2048, 2048, 16
QR, KVR = 1536, 512
NOPE, RD, VD = 128, 64, 128
QK_D = NOPE + RD
HL = 4          # heads per core
G = 4           # head groups (= cores per batch group)
QSH = QR // G   # 384 c_q rows per core
KSH = KVR // G  # 128 c_kv rows per core
PAN = 512       # panel width
P = 128
GR = 320        # gather rows (bf16 units): 192 (cq fp8) + 128 (ckv bf16)

_cache = {}


def _build_module(phases="ABCD"):
    import concourse.bacc as bacc
    import concourse.mybir as mybir
    import concourse.tile as tile

    dt = mybir.dt
    f32, bf16, f8 = dt.float32, dt.bfloat16, dt.float8e4
    AF = mybir.ActivationFunctionType
    DR = mybir.MatmulPerfMode.DoubleRow

    nc = bacc.Bacc("TRN2", target_bir_lowering=False, debug=False, num_devices=8)

    def inp(name, shape, dtype=bf16):
        return nc.dram_tensor(name, shape, dtype, kind="ExternalInput").ap()

    # all inputs arrive pre-arranged partition-major ([128, flat]) so every
    # load is a contiguous 2-D DMA (128 descriptors, no strided gather)
    xp = inp("xp", [P, 4 * 16 * PAN])       # x[b].T as [p, panel, kt, s]
    xp8 = inp("xp8", [P, 4 * 16 * PAN], f8)
    wqd8 = inp("wqd8", [P, 8 * 2 * QSH], f8)   # Wq_down.T slice * 64, [p,j,two,m]
    wkvd = inp("wkvd", [P, 16 * KSH])       # Wkv_down.T slice, [p,kt,m]
    wkr = inp("wkr", [P, 16 * RD])          # Wk_rope.T * s2, [p,kt,m]
    wqf = inp("wqf", [P, 6 * 2 * 768], f8)  # [Wq_up|Wq_rope].T * s2*64
    wku = inp("wku", [P, 4 * 512])          # Wk_up_g.T * s2, [p,kt,m]
    wvu = inp("wvu", [P, 4 * 512])          # Wv_up_g.T
    wo = inp("wo", [P, 4 * D])              # Wo[:, cols_g].T, [p,kt,m]
    cosb = inp("cosb", [P, S])              # bf16 rope tables, 4x partition-tiled
    sinb = inp("sinb", [P, S])
    masks = inp("masks", [P, G * PAN])      # multiplicative causal masks
    onc = inp("onc", [P, 1])                # ones column
    y = nc.dram_tensor("y", [S, D], bf16, kind="ExternalOutput").ap()

    KT_D = D // P      # 16 k-tiles over model dim
    KT_KV = KVR // P   # 4
    NP = S // PAN      # 4 panels
    GROUPS = [[0, 1, 2, 3], [4, 5, 6, 7]]
    INV64 = 1.0 / 64.0

    with tile.TileContext(nc) as tc:
        with (
            tc.tile_pool(name="res", bufs=1) as res,
            tc.tile_pool(name="panels", bufs=8) as panels,
            tc.tile_pool(name="work", bufs=2) as work,
            tc.tile_pool(name="dram", bufs=1, space="DRAM") as dram,
        ):
            # ---- SBUF residents ----------------------------------------
            # q/k fp8 layout [P, head, slot, S]: slot 0 = nope dims,
            # slot 1 rows 0:64 = roped rope dims, rows 64:128 zero.
            q_sb = res.tile([P, HL, 2, S], f8, tag="q")
            k_sb = res.tile([P, HL, 2, S], f8, tag="k")
            v_sb = res.tile([P, S // P, 512], bf16, tag="v")
            ao_sb = res.tile([P, HL, S], bf16, tag="ao")
            masks_sb = res.tile([P, G, PAN], bf16, tag="masks")
            onc_sb = res.tile([P, 1], bf16, tag="onc")
            cos_sb = res.tile([P, S], bf16, tag="cos")
            sin_sb = res.tile([P, S], bf16, tag="sin")

            # zero the unused upper rope rows so the fp8 matmul never sees
            # garbage (e4m3 NaN patterns would poison 0*NaN)
            nc.vector.memset(q_sb[64:128, :, 1, :], 0.0)
            nc.vector.memset(k_sb[64:128, :, 1, :], 0.0)
            nc.gpsimd.dma_start(
                masks_sb[:], masks.rearrange("p (j q) -> p j q", q=PAN)
            )
            nc.gpsimd.dma_start(onc_sb[:], onc[:])
            nc.gpsimd.dma_start(cos_sb[:], cosb[:])
            nc.gpsimd.dma_start(sin_sb[:], sinb[:])

            # ---- DRAM staging for the latent AllGather ------------------
            ag_in = [dram.tile([GR, PAN], bf16, tag=f"agi{n}", name=f"agi{n}")
                     for n in range(NP)]
            ag_out = [dram.tile([G * GR, PAN], bf16, tag=f"ago{n}",
                                name=f"ago{n}") for n in range(NP)]

            def f8rows(t):
                # [rows, PAN] bf16 -> [2*rows, PAN] fp8 view of same bytes
                return t[:].bitcast(f8).rearrange("r (two s) -> (r two) s",
                                                  s=PAN)

            def rope_block(dst64, src64, ns, b0):
                # dst [64, PAN] fp8 slot-1 rows; src [64, PAN] bf16 whose
                # halves sit at absolute partitions b0 and b0+32 (the BIR
                # verifier wants matching base partitions for SBUF inputs,
                # hence the partition-tiled cos/sin tables)
                cs0, cs1 = cos_sb[b0 : b0 + 32, ns], cos_sb[b0 + 32 : b0 + 64, ns]
                sn0, sn1 = sin_sb[b0 : b0 + 32, ns], sin_sb[b0 + 32 : b0 + 64, ns]
                t1 = work.tile([32, PAN], bf16, tag="rt1")
                t2 = work.tile([32, PAN], bf16, tag="rt2")
                nc.vector.tensor_mul(t1, src64[0:32, :], cs0)
                nc.vector.tensor_mul(t2, src64[32:64, :], sn1)
                nc.vector.tensor_sub(dst64[0:32, :], t1, t2)
                t3 = work.tile([32, PAN], bf16, tag="rt1")
                t4 = work.tile([32, PAN], bf16, tag="rt2")
                nc.vector.tensor_mul(t3, src64[32:64, :], cs1)
                nc.vector.tensor_mul(t4, src64[0:32, :], sn0)
                nc.vector.tensor_add(dst64[32:64, :], t3, t4)

            # ---- Phase A + B -------------------------------------------
            with (
                tc.tile_pool(name="pa", bufs=1) as pa,
                tc.tile_pool(name="pb", bufs=1) as pb,
                tc.tile_pool(name="bq", bufs=3) as bq,
                tc.tile_pool(name="psA", bufs=4, space="PSUM") as psA,
                tc.tile_pool(name="psB", bufs=3, space="PSUM") as psB,
            ):
                # A weights: wqd8 chunked so the first matmul starts early
                wqd_sb = pa.tile([P, KT_D // 2, 2, QSH], f8, tag="wqd8")
                x80_ch = []
                for c in range(4):
                    nc.sync.dma_start(wqd_sb[:, 2 * c : 2 * c + 2, :, :],
                                      wqd8[:, c * 2 * 2 * QSH : (c + 1) * 2 * 2 * QSH])
                    t = panels.tile([P, 2, 2, PAN], f8, tag="panel8",
                                    name=f"x8_sb0_{c}")
                    nc.sync.dma_start(t[:], xp8[:, c * 4 * PAN : (c + 1) * 4 * PAN])
                    x80_ch.append(t)
                wkvd_sb = pa.tile([P, KT_D, KSH], bf16, tag="wkvd")
                nc.sync.dma_start(wkvd_sb[:], wkvd[:])
                wkr_sb = pa.tile([P, KT_D, RD], bf16, tag="wkr")
                nc.sync.dma_start(wkr_sb[:], wkr[:])
                x0_ch = []
                for c in range(4):
                    t = panels.tile([P, 4, PAN], bf16, tag="panel",
                                    name=f"x_sb0_{c}")
                    (nc.sync if c % 2 == 0 else nc.scalar).dma_start(
                        t[:], xp[:, c * 4 * PAN : (c + 1) * 4 * PAN])
                    x0_ch.append(t)

                # B weights on the gpsimd DMA queue, off the hot SP queue
                def load_b_weights():
                    wqf_sb = pb.tile([P, 6, 2, 768], f8, tag="wqf")
                    nc.gpsimd.dma_start(wqf_sb[:], wqf[:])
                    wku_sb = pb.tile([P, KT_KV, 512], bf16, tag="wku")
                    nc.gpsimd.dma_start(wku_sb[:], wku[:])
                    wvu_sb = pb.tile([P, KT_KV, 512], bf16, tag="wvu")
                    nc.gpsimd.dma_start(wvu_sb[:], wvu[:])
                    return wqf_sb, wku_sb, wvu_sb

                def phase_a(n):
                    """my latent slices for panel n + local k_rope, then gather"""
                    ns = slice(n * PAN, (n + 1) * PAN)
                    if n == 0:
                        x_ch, x8_ch = x0_ch, x80_ch
                    else:
                        x8_ch = []
                        for c in range(4):
                            t = panels.tile([P, 2, 2, PAN], f8, tag="panel8",
                                            name=f"x8_sb{n}_{c}")
                            o = (n * 16 + c * 4) * PAN
                            nc.sync.dma_start(t[:], xp8[:, o : o + 4 * PAN])
                            x8_ch.append(t)
                        x_ch = []
                        for c in range(4):
                            t = panels.tile([P, 4, PAN], bf16, tag="panel",
                                            name=f"x_sb{n}_{c}")
                            o = (n * 16 + c * 4) * PAN
                            (nc.sync if c % 2 == 0 else nc.scalar).dma_start(
                                t[:], xp[:, o : o + 4 * PAN])
                            x_ch.append(t)
                    gin8 = f8rows(ag_in[n])
                    # c_q in fp8 DoubleRow: kt-pair-major over 3 concurrent
                    # psums so matmuls start as soon as the first chunk lands
                    pss = [psA.tile([P, PAN], f32, tag="psA", name=f"psA{n}_{m}")
                           for m in range(4)]
                    for j in range(KT_D // 2):
                        xk = x8_ch[j // 2][:, j % 2, :, :]
                        st0, st1 = (j == 0), (j == KT_D // 2 - 1)
                        for m in range(3):
                            nc.tensor.matmul(
                                pss[m],
                                lhsT=wqd_sb[:, j, :, m * P : (m + 1) * P],
                                rhs=xk, start=st0, stop=st1, perf_mode=DR,
                            )
                    # c_kv (bf16, feeds v) + local k_rope share the x chunks
                    ps_kr = psA.tile([64, PAN], f32, tag="psKR", bufs=1)
                    for kt in range(KT_D):
                        xk = x_ch[kt // 4][:, kt % 4, :]
                        st0, st1 = (kt == 0), (kt == KT_D - 1)
                        nc.tensor.matmul(pss[3], lhsT=wkvd_sb[:, kt, :],
                                         rhs=xk, start=st0, stop=st1)
                        nc.tensor.matmul(ps_kr, lhsT=wkr_sb[:, kt, :],
                                         rhs=xk, start=st0, stop=st1)
                    cqr = gin8[0:384, :].rearrange("(p r) s -> p r s", r=3)
                    for m in range(3):  # c_q -> fp8 staging (scale 1/64 back)
                        st = work.tile([P, PAN], f8, tag="cq8")
                        nc.scalar.activation(st, pss[m], AF.Copy, scale=INV64)
                        nc.sync.dma_start(cqr[:, m, :], st)
                    st = work.tile([P, PAN], bf16, tag="ckvst")
                    nc.scalar.activation(st, pss[3], AF.Copy)
                    nc.sync.dma_start(ag_in[n][192:320, :], st)
                    nc.gpsimd.collective_compute(
                        "AllGather", mybir.AluOpType.bypass,
                        replica_groups=GROUPS,
                        ins=[ag_in[n].opt()], outs=[ag_out[n].opt()],
                    )
                    krb = work.tile([64, PAN], bf16, tag="krb")
                    nc.scalar.activation(krb, ps_kr, AF.Copy)
                    rope_block(k_sb[0:64, 0, 1, ns], krb, ns, 0)
                    for hh in range(1, HL):  # replicate to the other heads
                        nc.vector.tensor_copy(k_sb[0:64, hh, 1, ns],
                                              k_sb[0:64, 0, 1, ns])

                def phase_b(n):
                    """up-projections for panel n from the gathered latents"""
                    ns = slice(n * PAN, (n + 1) * PAN)
                    gout8 = f8rows(ag_out[n])
                    cqf = bq.tile([P, 12, PAN], f8, tag="cqf")
                    nc.scalar.dma_start(
                        cqf[:],
                        gout8.rearrange("(gi x) s -> gi x s", x=2 * GR)[
                            :, 0:QSH, :
                        ].rearrange("gi (p r) s -> p gi r s", r=3),
                    )
                    ckv = bq.tile([P, KT_KV, PAN], bf16, tag="ckv")
                    nc.scalar.dma_start(
                        ckv[:],
                        ag_out[n].rearrange("(gi x) s -> gi x s", x=GR)[
                            :, 192:320, :
                        ].rearrange("gi p s -> p gi s"),
                    )
                    for m in range(HL):  # q nope heads (fp8 DoubleRow)
                        ps = psB.tile([P, PAN], f32, tag="psB")
                        for j in range(6):
                            nc.tensor.matmul(
                                ps,
                                lhsT=wqf_sb[:, j, :, m * P : (m + 1) * P],
                                rhs=cqf[:, 2 * j : 2 * j + 2, :],
                                start=(j == 0), stop=(j == 5), perf_mode=DR,
                            )
                        nc.scalar.activation(q_sb[:, m, 0, ns], ps, AF.Copy,
                                             scale=INV64)
                    for hp in range(HL // 2):  # rope: two heads per psum
                        c0 = 512 + 128 * hp
                        ps = psB.tile([P, PAN], f32, tag="psB")
                        for j in range(6):
                            nc.tensor.matmul(
                                ps,
                                lhsT=wqf_sb[:, j, :, c0 : c0 + 128],
                                rhs=cqf[:, 2 * j : 2 * j + 2, :],
                                start=(j == 0), stop=(j == 5), perf_mode=DR,
                            )
                        qrp = work.tile([P, PAN], bf16, tag="qrp")
                        nc.scalar.activation(qrp, ps, AF.Copy, scale=INV64)
                        rope_block(q_sb[0:64, 2 * hp, 1, ns], qrp[0:64, :], ns, 0)
                        rope_block(q_sb[0:64, 2 * hp + 1, 1, ns],
                                   qrp[64:128, :], ns, 64)
                    for m in range(HL):  # k_c
                        ps = psB.tile([P, PAN], f32, tag="psB")
                        for kt in range(KT_KV):
                            nc.tensor.matmul(
                                ps,
                                lhsT=wku_sb[:, kt, m * P : (m + 1) * P],
                                rhs=ckv[:, kt, :],
                                start=(kt == 0), stop=(kt == KT_KV - 1),
                            )
                        nc.scalar.activation(k_sb[:, m, 0, ns], ps, AF.Copy)
                    for sti in range(4):  # v for this panel's S-tiles
                        ps = psB.tile([P, PAN], f32, tag="psB")
                        for kt in range(KT_KV):
                            nc.tensor.matmul(
                                ps,
                                lhsT=ckv[:, kt, sti * P : (sti + 1) * P],
                                rhs=wvu_sb[:, kt, :],
                                start=(kt == 0), stop=(kt == KT_KV - 1),
                            )
                        nc.scalar.activation(v_sb[:, 4 * n + sti, :], ps, AF.Copy)

                phase_a(0)
                wqf_sb, wku_sb, wvu_sb = load_b_weights()
                phase_a(1)
                phase_a(2)
                phase_b(0)
                phase_a(3)
                phase_b(1)
                phase_b(2)
                phase_b(3)

            # ---------------- Phase C: SDPA + Phase D interleaved --------
            if "C" not in phases:
                # timing-partial build: consume B outputs so nothing is elided
                nc.gpsimd.dma_start(y[0:P, 0:PAN], q_sb[:, 0, 0, 0:PAN])
                nc.gpsimd.dma_start(y[P : 2 * P, 0:PAN], k_sb[:, 0, 0, 0:PAN])
                nc.gpsimd.dma_start(y[2 * P : 3 * P, 0:PAN], v_sb[:, 0, 0:PAN])
            elif True:
                do_cd(tc, nc, phases, q_sb, k_sb, v_sb, ao_sb, masks_sb,
                      onc_sb, wo, work, y)

    nc.compile()
    return nc


def do_cd(tc, nc, phases, q_sb, k_sb, v_sb, ao_sb, masks_sb, onc_sb, wo,
          work, y):
    import concourse.mybir as mybir

    dt = mybir.dt
    f32, bf16 = dt.float32, dt.bfloat16
    AF = mybir.ActivationFunctionType
    DR = mybir.MatmulPerfMode.DoubleRow
    if True:
        if True:
            with (
                tc.tile_pool(name="pw", bufs=1) as pw,
                tc.tile_pool(name="pe", bufs=26) as pe,
                tc.tile_pool(name="pds", bufs=2) as pds,
                tc.tile_pool(name="py", bufs=4) as py,
                tc.tile_pool(name="psS", bufs=2, space="PSUM") as psS,
                tc.tile_pool(name="psO", bufs=2, space="PSUM") as psO,
                tc.tile_pool(name="psDn", bufs=1, space="PSUM") as psDn,
                tc.tile_pool(name="psD", bufs=3, space="PSUM") as psD,
            ):
                wo_sb = pw.tile([P, HL, D], bf16, tag="wo")
                nc.gpsimd.dma_start(wo_sb[:], wo[:])
                ycnt = [0]

                def d_unit(m, nh, tail=False):
                    """one output-projection unit: 2 nn-blocks of row-tile m"""
                    ms = slice(m * P, (m + 1) * P)
                    yst = py.tile([P, 2 * PAN], bf16, tag="yst")
                    for sub in range(2):
                        nn = 2 * nh + sub
                        ps = psD.tile([P, PAN], f32, tag="psD")
                        for kt in range(HL):
                            nc.tensor.matmul(
                                ps,
                                lhsT=ao_sb[:, kt, ms],
                                rhs=wo_sb[:, kt, nn * PAN : (nn + 1) * PAN],
                                start=(kt == 0), stop=(kt == HL - 1),
                            )
                        nc.vector.tensor_copy(
                            yst[:, sub * PAN : (sub + 1) * PAN], ps)
                    q = (nc.sync, nc.gpsimd, nc.scalar)[ycnt[0] % 3]
                    ycnt[0] += 1
                    q.dma_start(y[ms, 2 * nh * PAN : (2 * nh + 2) * PAN], yst)

                dq = []  # pending D units, drained inside the SDPA stream

                def d_fill():
                    if dq and "D" in phases:
                        m, nh = dq.pop(0)
                        d_unit(m, nh)

                def sdpa_pair(g, h0):
                    """two heads' SDPA chains interleaved kb-by-kb so the
                    in-order PE queue always has independent ready work while
                    the other chain waits on its exp"""
                    gs = slice(g * PAN, (g + 1) * PAN)
                    nk = 4 * (g + 1)
                    hs = (h0, h0 + 1)
                    ps_o = {h: psO.tile([P, PAN], f32, tag="ps_o",
                                        name=f"pso{g}_{h}") for h in hs}
                    # both chains' denominators live in one psum bank, at
                    # partition bases 0 and 32 (legal matmul tile positions)
                    ps_d2 = psDn.tile([33, PAN], f32, tag="ps_d")
                    e_t = {h: [None] * nk for h in hs}
                    dsum = {h: [None] * (nk // 2) for h in hs}
                    qsum = {h: [None] * (nk // 4) for h in hs}
                    pend = {h: [] for h in hs}

                    def consume(ci, h, kb):
                        nc.tensor.matmul(
                            ps_o[h], lhsT=v_sb[:, kb, h * P : (h + 1) * P],
                            rhs=e_t[h][kb],
                            start=(kb == 0), stop=(kb == nk - 1),
                        )
                        if kb % 4 == 3:
                            nc.tensor.matmul(
                                ps_d2[32 * ci : 32 * ci + 1, :],
                                lhsT=onc_sb[:], rhs=qsum[h][kb // 4],
                                start=(kb == 3), stop=(kb == nk - 1),
                                skip_group_check=True,
                            )

                    for kb in range(nk):
                        ks = slice(kb * P, (kb + 1) * P)
                        for ci, h in enumerate(hs):
                            ps_s = psS.tile([P, PAN], f32, tag="ps_s")
                            nc.tensor.matmul(
                                ps_s, lhsT=k_sb[:, h, :, ks],
                                rhs=q_sb[:, h, :, gs],
                                start=True, stop=True, perf_mode=DR,
                            )
                            e = pe.tile([P, PAN], bf16, tag="e")
                            nc.scalar.activation(e, ps_s, AF.Exp)
                            if kb >= 4 * g:
                                nc.vector.tensor_mul(
                                    e, e, masks_sb[:, kb - 4 * g, :])
                            e_t[h][kb] = e
                            if kb % 2 == 1:
                                ds = pds.tile([P, PAN], bf16, tag=f"dsum{ci}")
                                nc.vector.tensor_add(ds, e_t[h][kb - 1],
                                                     e_t[h][kb])
                                dsum[h][kb // 2] = ds
                            if kb % 4 == 3:
                                qs = pds.tile([P, PAN], bf16, tag=f"qsum{ci}")
                                nc.vector.tensor_add(qs, dsum[h][kb // 2 - 1],
                                                     dsum[h][kb // 2])
                                qsum[h][kb // 4] = qs
                            pend[h].append(kb)
                            if len(pend[h]) > 12:
                                consume(ci, h, pend[h].pop(0))
                    for ci, h in enumerate(hs):
                        while pend[h]:
                            consume(ci, h, pend[h].pop(0))
                    for ci, h in enumerate(hs):
                        rc = work.tile([1, PAN], f32, tag="rc")
                        nc.vector.reciprocal(rc, ps_d2[32 * ci : 32 * ci + 1, :])
                        bb = work.tile([P, PAN], f32, tag="bb")
                        nc.gpsimd.partition_broadcast(bb, rc)
                        nc.vector.tensor_mul(ao_sb[:, h, gs], ps_o[h], bb)

                for g in range(G):
                    if g >= 1:
                        dq.extend((4 * (g - 1) + mm, nh)
                                  for mm in range(4) for nh in range(2))
                    for hp in range(HL // 2):
                        sdpa_pair(g, 2 * hp)
                        for _ in range(4):
                            d_fill()
                    while dq:
                        d_fill()
                if "D" in phases:
                    for m in range(4 * (G - 1), 4 * G):
                        for nh in range(2):
                            d_unit(m, nh, tail=True)


def _prep_inputs(x, positions, Wq_down, Wq_up, Wq_rope, Wkv_down, Wk_up, Wv_up,
                 Wk_rope, Wo):
    s2 = np.float32(QK_D ** -0.25)  # sqrt of the 1/sqrt(qk_d) scale
    bf = lambda a: np.ascontiguousarray(a).astype(BF16)

    def pmaj(a, kt):
        # [kt*128, m] -> [128, kt*m] partition-major
        m = a.shape[1]
        return np.ascontiguousarray(
            a.reshape(kt, P, m).transpose(1, 0, 2).reshape(P, kt * m))

    inv_freq = 1.0 / (10000.0 ** (np.arange(0, RD, 2, dtype=np.float32) / RD))
    ang = positions.astype(np.float32)[:, None] * inv_freq  # (S, 32)
    shared = {
        "wkr": pmaj(Wk_rope.T * s2, 16).astype(BF16),
        "onc": np.ones((P, 1), BF16),
        "cosb": bf(np.tile(np.cos(ang).T, (4, 1))),
        "sinb": bf(np.tile(np.sin(ang).T, (4, 1))),
    }

    mk = np.zeros((P, G * PAN), np.float32)
    for j in range(G):
        p = np.arange(P)[:, None]
        q = np.arange(PAN)[None, :]
        mk[:, j * PAN : (j + 1) * PAN] = (j * P + p <= q).astype(np.float32)
    shared["masks"] = mk.astype(BF16)

    wqdT = Wq_down.T  # (D, QR)
    wkvdT = Wkv_down.T  # (D, KVR)
    per_g = []
    for g in range(G):
        rs, rr = slice(512 * g, 512 * (g + 1)), slice(256 * g, 256 * (g + 1))
        wqall = np.concatenate([Wq_up[rs].T, Wq_rope[rr].T], axis=1)
        per_g.append({
            "wqd8": pmaj(wqdT[:, QSH * g : QSH * (g + 1)] * 64.0, 16).astype(F8),
            "wkvd": pmaj(wkvdT[:, KSH * g : KSH * (g + 1)], 16).astype(BF16),
            "wqf": pmaj(wqall * (s2 * 64.0), 12).astype(F8),
            "wku": pmaj(Wk_up[rs].T * s2, 4).astype(BF16),
            "wvu": pmaj(Wv_up[rs].T, 4).astype(BF16),
            "wo": pmaj(Wo[:, rs].T, 4).astype(BF16),
        })
    # x as [p, panel, kt, s] partition-major flat
    def xprep(a):
        # a: [D, S] -> [128, NP*16*PAN]
        return np.ascontiguousarray(
            a.reshape(16, P, 4, PAN).transpose(1, 2, 0, 3).reshape(P, -1))

    xTs = [x[b].T for b in range(B)]
    xps = [xprep(a).astype(BF16) for a in xTs]
    xp8s = [xprep(a).astype(F8) for a in xTs]

    in_maps = []
    for c in range(8):
        b, g = c // G, c % G
        m = dict(shared)
        m.update(per_g[g])
        m["xp"] = xps[b]
        m["xp8"] = xp8s[b]
        in_maps.append(m)
    return in_maps


def kernel(**inputs):
    from concourse.bass_utils import run_bass_kernel_spmd

    if "nc" not in _cache:
        _cache["nc"] = _build_module()
    nc = _cache["nc"]

    in_maps = _prep_inputs(**inputs)
    res = None
    for attempt in range(3):
        try:
            res = run_bass_kernel_spmd(nc, in_maps, core_ids=list(range(8)))
            break
        except Exception:
            if attempt == 2:
                raise
    out = np.zeros((B, S, D), np.float32)
    for c in range(8):
        out[c // G] += res.results[c]["y"].astype(np.float32)
    return out


# revision 97
# speedup vs baseline: 1.0119x; 1.0003x over previous
"""Trainium2 Bass kernel for MultiLatentAttention (MLA).

Sharding: 8 cores = 2 (batch) x 4 (head-groups of 4 heads).
Within each batch group of 4 cores, the down-projections are sharded by
output rows and AllGathered (per S-panel, pipelined).  Each core computes
the shared k_rope head locally for all panels (no collective), runs its
4 heads' up-projections + SDPA and a partial output projection
y_part = attn_out @ Wo[:, heads].T.  Host sums the 4 partials per batch.

Speed tricks vs the bf16 baseline:
- Scores are computed with fp8e4 DoubleRow matmuls: the 192-dim qk
  contraction (128 nope + 64 rope) is packed as [128, 2, *] operands and
  runs at 0.5 cycles/column (4x fewer PE cycles than 2 bf16 matmuls).
- The q up-projection also runs in fp8 DoubleRow (k-tile pairs), with
  the fp8 weights pre-scaled by 64 to stay in e4m3 normal range; the 64
  is divided back out in the psum->q cast on the Act engine.
- c_q travels through the AllGather in fp8 (mixed fp8/bf16 gather buffer
  via bitcast views), shrinking collective bytes and killing the
  readback cast.
- Softmax denominators use DVE pair-sums of e-tiles, halving the
  ones-matmul count; attention output stays in SBUF (no DRAM roundtrip)
  and the output projection is software-pipelined into the SDPA stream.
- 1/sqrt(192) is split as s2 = 192**-0.25 folded into both the q-side
  and k-side weights so fp8 operands are magnitude-balanced.
All fp8/bf16 matmuls accumulate in f32 PSUM; y partials are f32.
"""

import sys

if "/opt/trn_rl_repo" not in sys.path:
    sys.path.insert(0, "/opt/trn_rl_repo")

import numpy as np
import ml_dtypes

BF16 = ml_dtypes.bfloat16
F8 = ml_dtypes.float8_e4m3

B, S, D, H = 2, 2048, 2048, 16
QR, KVR = 1536, 512
NOPE, RD, VD = 128, 64, 128
QK_D = NOPE + RD
HL = 4          # heads per core
G = 4           # head groups (= cores per batch group)
QSH = QR // G   # 384 c_q rows per core
KSH = KVR // G  # 128 c_kv rows per core
PAN = 512       # panel width
P = 128
GR = 320        # gather rows (bf16 units): 192 (cq fp8) + 128 (ckv bf16)

_cache = {}


def _build_module(phases="ABCD"):
    import concourse.bacc as bacc
    import concourse.mybir as mybir
    import concourse.tile as tile

    dt = mybir.dt
    f32, bf16, f8 = dt.float32, dt.bfloat16, dt.float8e4
    AF = mybir.ActivationFunctionType
    DR = mybir.MatmulPerfMode.DoubleRow

    nc = bacc.Bacc("TRN2", target_bir_lowering=False, debug=False, num_devices=8)

    def inp(name, shape, dtype=bf16):
        return nc.dram_tensor(name, shape, dtype, kind="ExternalInput").ap()

    # all inputs arrive pre-arranged partition-major ([128, flat]) so every
    # load is a contiguous 2-D DMA (128 descriptors, no strided gather)
    xp = inp("xp", [P, 4 * 16 * PAN])       # x[b].T as [p, panel, kt, s]
    xp8 = inp("xp8", [P, 4 * 16 * PAN], f8)
    wqd8 = inp("wqd8", [P, 8 * 2 * QSH], f8)   # Wq_down.T slice * 64, [p,j,two,m]
    wkvd = inp("wkvd", [P, 16 * KSH])       # Wkv_down.T slice, [p,kt,m]
    wkr = inp("wkr", [P, 16 * RD])          # Wk_rope.T * s2, [p,kt,m]
    wqf = inp("wqf", [P, 6 * 2 * 768], f8)  # [Wq_up|Wq_rope].T * s2*64
    wku = inp("wku", [P, 4 * 512])          # Wk_up_g.T * s2, [p,kt,m]
    wvu = inp("wvu", [P, 4 * 512])          # Wv_up_g.T
    wo = inp("wo", [P, 4 * D])              # Wo[:, cols_g].T, [p,kt,m]
    cosb = inp("cosb", [P, S])              # bf16 rope tables, 4x partition-tiled
    sinb = inp("sinb", [P, S])
    masks = inp("masks", [P, G * PAN])      # multiplicative causal masks
    onc = inp("onc", [P, 1])                # ones column
    y = nc.dram_tensor("y", [S, D], bf16, kind="ExternalOutput").ap()

    KT_D = D // P      # 16 k-tiles over model dim
    KT_KV = KVR // P   # 4
    NP = S // PAN      # 4 panels
    GROUPS = [[0, 1, 2, 3], [4, 5, 6, 7]]
    INV64 = 1.0 / 64.0

    with tile.TileContext(nc) as tc:
        with (
            tc.tile_pool(name="res", bufs=1) as res,
            tc.tile_pool(name="panels", bufs=8) as panels,
            tc.tile_pool(name="work", bufs=2) as work,
            tc.tile_pool(name="dram", bufs=1, space="DRAM") as dram,
        ):
            # ---- SBUF residents ----------------------------------------
            # q/k fp8 layout [P, head, slot, S]: slot 0 = nope dims,
            # slot 1 rows 0:64 = roped rope dims, rows 64:128 zero.
            q_sb = res.tile([P, HL, 2, S], f8, tag="q")
            k_sb = res.tile([P, HL, 2, S], f8, tag="k")
            v_sb = res.tile([P, S // P, 512], bf16, tag="v")
            ao_sb = res.tile([P, HL, S], bf16, tag="ao")
            masks_sb = res.tile([P, G, PAN], bf16, tag="masks")
            onc_sb = res.tile([P, 1], bf16, tag="onc")
            cos_sb = res.tile([P, S], bf16, tag="cos")
            sin_sb = res.tile([P, S], bf16, tag="sin")

            # zero the unused upper rope rows so the fp8 matmul never sees
            # garbage (e4m3 NaN patterns would poison 0*NaN)
            nc.vector.memset(q_sb[64:128, :, 1, :], 0.0)
            nc.vector.memset(k_sb[64:128, :, 1, :], 0.0)
            nc.gpsimd.dma_start(
                masks_sb[:], masks.rearrange("p (j q) -> p j q", q=PAN)
            )
            nc.gpsimd.dma_start(onc_sb[:], onc[:])
            nc.gpsimd.dma_start(cos_sb[:], cosb[:])
            nc.gpsimd.dma_start(sin_sb[:], sinb[:])

            # ---- DRAM staging for the latent AllGather ------------------
            ag_in = [dram.tile([GR, PAN], bf16, tag=f"agi{n}", name=f"agi{n}")
                     for n in range(NP)]
            ag_out = [dram.tile([G * GR, PAN], bf16, tag=f"ago{n}",
                                name=f"ago{n}") for n in range(NP)]

            def f8rows(t):
                # [rows, PAN] bf16 -> [2*rows, PAN] fp8 view of same bytes
                return t[:].bitcast(f8).rearrange("r (two s) -> (r two) s",
                                                  s=PAN)

            def rope_block(dst64, src64, ns, b0):
                # dst [64, PAN] fp8 slot-1 rows; src [64, PAN] bf16 whose
                # halves sit at absolute partitions b0 and b0+32 (the BIR
                # verifier wants matching base partitions for SBUF inputs,
                # hence the partition-tiled cos/sin tables)
                cs0, cs1 = cos_sb[b0 : b0 + 32, ns], cos_sb[b0 + 32 : b0 + 64, ns]
                sn0, sn1 = sin_sb[b0 : b0 + 32, ns], sin_sb[b0 + 32 : b0 + 64, ns]
                t1 = work.tile([32, PAN], bf16, tag="rt1")
                t2 = work.tile([32, PAN], bf16, tag="rt2")
                nc.vector.tensor_mul(t1, src64[0:32, :], cs0)
                nc.vector.tensor_mul(t2, src64[32:64, :], sn1)
                nc.vector.tensor_sub(dst64[0:32, :], t1, t2)
                t3 = work.tile([32, PAN], bf16, tag="rt1")
                t4 = work.tile([32, PAN], bf16, tag="rt2")
                nc.vector.tensor_mul(t3, src64[32:64, :], cs1)
                nc.vector.tensor_mul(t4, src64[0:32, :], sn0)
                nc.vector.tensor_add(dst64[32:64, :], t3, t4)

            # ---- Phase A + B -------------------------------------------
            with (
                tc.tile_pool(name="pa", bufs=1) as pa,
                tc.tile_pool(name="pb", bufs=1) as pb,
                tc.tile_pool(name="bq", bufs=3) as bq,
                tc.tile_pool(name="psA", bufs=4, space="PSUM") as psA,
                tc.tile_pool(name="psB", bufs=3, space="PSUM") as psB,
            ):
                # A weights: wqd8 chunked so the first matmul starts early
                wqd_sb = pa.tile([P, KT_D // 2, 2, QSH], f8, tag="wqd8")
                x80_ch = []
                for c in range(4):
                    nc.sync.dma_start(wqd_sb[:, 2 * c : 2 * c + 2, :, :],
                                      wqd8[:, c * 2 * 2 * QSH : (c + 1) * 2 * 2 * QSH])
                    t = panels.tile([P, 2, 2, PAN], f8, tag="panel8",
                                    name=f"x8_sb0_{c}")
                    nc.sync.dma_start(t[:], xp8[:, c * 4 * PAN : (c + 1) * 4 * PAN])
                    x80_ch.append(t)
                wkvd_sb = pa.tile([P, KT_D, KSH], bf16, tag="wkvd")
                nc.sync.dma_start(wkvd_sb[:], wkvd[:])
                wkr_sb = pa.tile([P, KT_D, RD], bf16, tag="wkr")
                nc.sync.dma_start(wkr_sb[:], wkr[:])
                x0_ch = []
                for c in range(4):
                    t = panels.tile([P, 4, PAN], bf16, tag="panel",
                                    name=f"x_sb0_{c}")
                    (nc.sync if c % 2 == 0 else nc.scalar).dma_start(
                        t[:], xp[:, c * 4 * PAN : (c + 1) * 4 * PAN])
                    x0_ch.append(t)

                # B weights on the gpsimd DMA queue, off the hot SP queue
                def load_b_weights():
                    wqf_sb = pb.tile([P, 6, 2, 768], f8, tag="wqf")
                    nc.gpsimd.dma_start(wqf_sb[:], wqf[:])
                    wku_sb = pb.tile([P, KT_KV, 512], bf16, tag="wku")
                    nc.gpsimd.dma_start(wku_sb[:], wku[:])
                    wvu_sb = pb.tile([P, KT_KV, 512], bf16, tag="wvu")
                    nc.gpsimd.dma_start(wvu_sb[:], wvu[:])
                    return wqf_sb, wku_sb, wvu_sb

                def phase_a(n):
                    """my latent slices for panel n + local k_rope, then gather"""
                    ns = slice(n * PAN, (n + 1) * PAN)
                    if n == 0:
                        x_ch, x8_ch = x0_ch, x80_ch
                    else:
                        x8_ch = []
                        for c in range(4):
                            t = panels.tile([P, 2, 2, PAN], f8, tag="panel8",
                                            name=f"x8_sb{n}_{c}")
                            o = (n * 16 + c * 4) * PAN
                            nc.sync.dma_start(t[:], xp8[:, o : o + 4 * PAN])
                            x8_ch.append(t)
                        x_ch = []
                        for c in range(4):
                            t = panels.tile([P, 4, PAN], bf16, tag="panel",
                                            name=f"x_sb{n}_{c}")
                            o = (n * 16 + c * 4) * PAN
                            (nc.sync if c % 2 == 0 else nc.scalar).dma_start(
                                t[:], xp[:, o : o + 4 * PAN])
                            x_ch.append(t)
                    gin8 = f8rows(ag_in[n])
                    # c_q in fp8 DoubleRow: kt-pair-major over 3 concurrent
                    # psums so matmuls start as soon as the first chunk lands
                    pss = [psA.tile([P, PAN], f32, tag="psA", name=f"psA{n}_{m}")
                           for m in range(4)]
                    for j in range(KT_D // 2):
                        xk = x8_ch[j // 2][:, j % 2, :, :]
                        st0, st1 = (j == 0), (j == KT_D // 2 - 1)
                        for m in range(3):
                            nc.tensor.matmul(
                                pss[m],
                                lhsT=wqd_sb[:, j, :, m * P : (m + 1) * P],
                                rhs=xk, start=st0, stop=st1, perf_mode=DR,
                            )
                    # c_kv (bf16, feeds v) + local k_rope share the x chunks
                    ps_kr = psA.tile([64, PAN], f32, tag="psKR", bufs=1)
                    for kt in range(KT_D):
                        xk = x_ch[kt // 4][:, kt % 4, :]
                        st0, st1 = (kt == 0), (kt == KT_D - 1)
                        nc.tensor.matmul(pss[3], lhsT=wkvd_sb[:, kt, :],
                                         rhs=xk, start=st0, stop=st1)
                        nc.tensor.matmul(ps_kr, lhsT=wkr_sb[:, kt, :],
                                         rhs=xk, start=st0, stop=st1)
                    cqr = gin8[0:384, :].rearrange("(p r) s -> p r s", r=3)
                    for m in range(3):  # c_q -> fp8 staging (scale 1/64 back)
                        st = work.tile([P, PAN], f8, tag="cq8")
                        nc.scalar.activation(st, pss[m], AF.Copy, scale=INV64)
                        nc.sync.dma_start(cqr[:, m, :], st)
                    st = work.tile([P, PAN], bf16, tag="ckvst")
                    nc.scalar.activation(st, pss[3], AF.Copy)
                    nc.sync.dma_start(ag_in[n][192:320, :], st)
                    nc.gpsimd.collective_compute(
                        "AllGather", mybir.AluOpType.bypass,
                        replica_groups=GROUPS,
                        ins=[ag_in[n].opt()], outs=[ag_out[n].opt()],
                    )
                    krb = work.tile([64, PAN], bf16, tag="krb")
                    nc.scalar.activation(krb, ps_kr, AF.Copy)
                    rope_block(k_sb[0:64, 0, 1, ns], krb, ns, 0)
                    for hh in range(1, HL):  # replicate to the other heads
                        nc.vector.tensor_copy(k_sb[0:64, hh, 1, ns],
                                              k_sb[0:64, 0, 1, ns])

                def phase_b(n):
                    """up-projections for panel n from the gathered latents"""
                    ns = slice(n * PAN, (n + 1) * PAN)
                    gout8 = f8rows(ag_out[n])
                    cqf = bq.tile([P, 12, PAN], f8, tag="cqf")
                    nc.scalar.dma_start(
                        cqf[:],
                        gout8.rearrange("(gi x) s -> gi x s", x=2 * GR)[
                            :, 0:QSH, :
                        ].rearrange("gi (p r) s -> p gi r s", r=3),
                    )
                    ckv = bq.tile([P, KT_KV, PAN], bf16, tag="ckv")
                    nc.scalar.dma_start(
                        ckv[:],
                        ag_out[n].rearrange("(gi x) s -> gi x s", x=GR)[
                            :, 192:320, :
                        ].rearrange("gi p s -> p gi s"),
                    )
                    for m in range(HL):  # q nope heads (fp8 DoubleRow)
                        ps = psB.tile([P, PAN], f32, tag="psB")
                        for j in range(6):
                            nc.tensor.matmul(
                                ps,
                                lhsT=wqf_sb[:, j, :, m * P : (m + 1) * P],
                                rhs=cqf[:, 2 * j : 2 * j + 2, :],
                                start=(j == 0), stop=(j == 5), perf_mode=DR,
                            )
                        nc.scalar.activation(q_sb[:, m, 0, ns], ps, AF.Copy,
                                             scale=INV64)
                    for hp in range(HL // 2):  # rope: two heads per psum
                        c0 = 512 + 128 * hp
                        ps = psB.tile([P, PAN], f32, tag="psB")
                        for j in range(6):
                            nc.tensor.matmul(
                                ps,
                                lhsT=wqf_sb[:, j, :, c0 : c0 + 128],
                                rhs=cqf[:, 2 * j : 2 * j + 2, :],
                                start=(j == 0), stop=(j == 5), perf_mode=DR,
                            )
                        qrp = work.tile([P, PAN], bf16, tag="qrp")
                        nc.scalar.activation(qrp, ps, AF.Copy, scale=INV64)
                        rope_block(q_sb[0:64, 2 * hp, 1, ns], qrp[0:64, :], ns, 0)
                        rope_block(q_sb[0:64, 2 * hp + 1, 1, ns],
                                   qrp[64:128, :], ns, 64)
                    for m in range(HL):  # k_c
                        ps = psB.tile([P, PAN], f32, tag="psB")
                        for kt in range(KT_KV):
                            nc.tensor.matmul(
                                ps,
                                lhsT=wku_sb[:, kt, m * P : (m + 1) * P],
                                rhs=ckv[:, kt, :],
                                start=(kt == 0), stop=(kt == KT_KV - 1),
                            )
                        nc.scalar.activation(k_sb[:, m, 0, ns], ps, AF.Copy)
                    for sti in range(4):  # v for this panel's S-tiles
                        ps = psB.tile([P, PAN], f32, tag="psB")
                        for kt in range(KT_KV):
                            nc.tensor.matmul(
                                ps,
                                lhsT=ckv[:, kt, sti * P : (sti + 1) * P],
                                rhs=wvu_sb[:, kt, :],
                                start=(kt == 0), stop=(kt == KT_KV - 1),
                            )
                        nc.scalar.activation(v_sb[:, 4 * n + sti, :], ps, AF.Copy)

                phase_a(0)
                wqf_sb, wku_sb, wvu_sb = load_b_weights()
                phase_a(1)
                phase_a(2)
                phase_b(0)
                phase_a(3)
                phase_b(1)
                phase_b(2)
                phase_b(3)

            # ---------------- Phase C: SDPA + Phase D interleaved --------
            if "C" not in phases:
                # timing-partial build: consume B outputs so nothing is elided
                nc.gpsimd.dma_start(y[0:P, 0:PAN], q_sb[:, 0, 0, 0:PAN])
                nc.gpsimd.dma_start(y[P : 2 * P, 0:PAN], k_sb[:, 0, 0, 0:PAN])
                nc.gpsimd.dma_start(y[2 * P : 3 * P, 0:PAN], v_sb[:, 0, 0:PAN])
            elif True:
                do_cd(tc, nc, phases, q_sb, k_sb, v_sb, ao_sb, masks_sb,
                      onc_sb, wo, work, y)

    nc.compile()
    return nc


def do_cd(tc, nc, phases, q_sb, k_sb, v_sb, ao_sb, masks_sb, onc_sb, wo,
          work, y):
    import concourse.mybir as mybir

    dt = mybir.dt
    f32, bf16 = dt.float32, dt.bfloat16
    AF = mybir.ActivationFunctionType
    DR = mybir.MatmulPerfMode.DoubleRow
    if True:
        if True:
            with (
                tc.tile_pool(name="pw", bufs=1) as pw,
                tc.tile_pool(name="pe", bufs=26) as pe,
                tc.tile_pool(name="pds", bufs=2) as pds,
                tc.tile_pool(name="py", bufs=4) as py,
                tc.tile_pool(name="psS", bufs=3, space="PSUM") as psS,
                tc.tile_pool(name="psO", bufs=2, space="PSUM") as psO,
                tc.tile_pool(name="psDn", bufs=1, space="PSUM") as psDn,
                tc.tile_pool(name="psD", bufs=2, space="PSUM") as psD,
            ):
                wo_sb = pw.tile([P, HL, D], bf16, tag="wo")
                nc.gpsimd.dma_start(wo_sb[:], wo[:])
                ycnt = [0]

                def d_unit(m, nh, tail=False):
                    """one output-projection unit: 2 nn-blocks of row-tile m"""
                    ms = slice(m * P, (m + 1) * P)
                    yst = py.tile([P, 2 * PAN], bf16, tag="yst")
                    for sub in range(2):
                        nn = 2 * nh + sub
                        ps = psD.tile([P, PAN], f32, tag="psD")
                        for kt in range(HL):
                            nc.tensor.matmul(
                                ps,
                                lhsT=ao_sb[:, kt, ms],
                                rhs=wo_sb[:, kt, nn * PAN : (nn + 1) * PAN],
                                start=(kt == 0), stop=(kt == HL - 1),
                            )
                        nc.vector.tensor_copy(
                            yst[:, sub * PAN : (sub + 1) * PAN], ps)
                    q = (nc.sync, nc.gpsimd, nc.scalar)[ycnt[0] % 3]
                    ycnt[0] += 1
                    q.dma_start(y[ms, 2 * nh * PAN : (2 * nh + 2) * PAN], yst)

                dq = []  # pending D units, drained inside the SDPA stream

                def d_fill():
                    if dq and "D" in phases:
                        m, nh = dq.pop(0)
                        d_unit(m, nh)

                def sdpa_pair(g, h0):
                    """two heads' SDPA chains interleaved kb-by-kb so the
                    in-order PE queue always has independent ready work while
                    the other chain waits on its exp"""
                    gs = slice(g * PAN, (g + 1) * PAN)
                    nk = 4 * (g + 1)
                    hs = (h0, h0 + 1)
                    ps_o = {h: psO.tile([P, PAN], f32, tag="ps_o",
                                        name=f"pso{g}_{h}") for h in hs}
                    # both chains' denominators live in one psum bank, at
                    # partition bases 0 and 32 (legal matmul tile positions)
                    ps_d2 = psDn.tile([33, PAN], f32, tag="ps_d")
                    e_t = {h: [None] * nk for h in hs}
                    dsum = {h: [None] * (nk // 2) for h in hs}
                    qsum = {h: [None] * (nk // 4) for h in hs}
                    pend = {h: [] for h in hs}

                    def consume(ci, h, kb):
                        nc.tensor.matmul(
                            ps_o[h], lhsT=v_sb[:, kb, h * P : (h + 1) * P],
                            rhs=e_t[h][kb],
                            start=(kb == 0), stop=(kb == nk - 1),
                        )
                        if kb % 4 == 3:
                            nc.tensor.matmul(
                                ps_d2[32 * ci : 32 * ci + 1, :],
                                lhsT=onc_sb[:], rhs=qsum[h][kb // 4],
                                start=(kb == 3), stop=(kb == nk - 1),
                                skip_group_check=True,
                            )

                    for kb in range(nk):
                        ks = slice(kb * P, (kb + 1) * P)
                        for ci, h in enumerate(hs):
                            ps_s = psS.tile([P, PAN], f32, tag="ps_s")
                            nc.tensor.matmul(
                                ps_s, lhsT=k_sb[:, h, :, ks],
                                rhs=q_sb[:, h, :, gs],
                                start=True, stop=True, perf_mode=DR,
                            )
                            e = pe.tile([P, PAN], bf16, tag="e")
                            nc.scalar.activation(e, ps_s, AF.Exp)
                            if kb >= 4 * g:
                                nc.vector.tensor_mul(
                                    e, e, masks_sb[:, kb - 4 * g, :])
                            e_t[h][kb] = e
                            if kb % 2 == 1:
                                ds = pds.tile([P, PAN], bf16, tag=f"dsum{ci}")
                                nc.vector.tensor_add(ds, e_t[h][kb - 1],
                                                     e_t[h][kb])
                                dsum[h][kb // 2] = ds
                            if kb % 4 == 3:
                                qs = pds.tile([P, PAN], bf16, tag=f"qsum{ci}")
                                nc.vector.tensor_add(qs, dsum[h][kb // 2 - 1],
                                                     dsum[h][kb // 2])
                                qsum[h][kb // 4] = qs
                            pend[h].append(kb)
                            if len(pend[h]) > 12:
                                consume(ci, h, pend[h].pop(0))
                    for ci, h in enumerate(hs):
                        while pend[h]:
                            consume(ci, h, pend[h].pop(0))
                    for ci, h in enumerate(hs):
                        rc = work.tile([1, PAN], f32, tag="rc")
                        nc.vector.reciprocal(rc, ps_d2[32 * ci : 32 * ci + 1, :])
                        bb = work.tile([P, PAN], f32, tag="bb")
                        nc.gpsimd.partition_broadcast(bb, rc)
                        nc.vector.tensor_mul(ao_sb[:, h, gs], ps_o[h], bb)

                for g in range(G):
                    if g >= 1:
                        dq.extend((4 * (g - 1) + mm, nh)
                                  for mm in range(4) for nh in range(2))
                    for hp in range(HL // 2):
                        sdpa_pair(g, 2 * hp)
                        for _ in range(4):
                            d_fill()
                    while dq:
                        d_fill()
                if "D" in phases:
                    for m in range(4 * (G - 1), 4 * G):
                        for nh in range(2):
                            d_unit(m, nh, tail=True)


def _prep_inputs(x, positions, Wq_down, Wq_up, Wq_rope, Wkv_down, Wk_up, Wv_up,
                 Wk_rope, Wo):
    s2 = np.float32(QK_D ** -0.25)  # sqrt of the 1/sqrt(qk_d) scale
    bf = lambda a: np.ascontiguousarray(a).astype(BF16)

    def pmaj(a, kt):
        # [kt*128, m] -> [128, kt*m] partition-major
        m = a.shape[1]
        return np.ascontiguousarray(
            a.reshape(kt, P, m).transpose(1, 0, 2).reshape(P, kt * m))

    inv_freq = 1.0 / (10000.0 ** (np.arange(0, RD, 2, dtype=np.float32) / RD))
    ang = positions.astype(np.float32)[:, None] * inv_freq  # (S, 32)
    shared = {
        "wkr": pmaj(Wk_rope.T * s2, 16).astype(BF16),
        "onc": np.ones((P, 1), BF16),
        "cosb": bf(np.tile(np.cos(ang).T, (4, 1))),
        "sinb": bf(np.tile(np.sin(ang).T, (4, 1))),
    }

    mk = np.zeros((P, G * PAN), np.float32)
    for j in range(G):
        p = np.arange(P)[:, None]
        q = np.arange(PAN)[None, :]
        mk[:, j * PAN : (j + 1) * PAN] = (j * P + p <= q).astype(np.float32)
    shared["masks"] = mk.astype(BF16)

    wqdT = Wq_down.T  # (D, QR)
    wkvdT = Wkv_down.T  # (D, KVR)
    per_g = []
    for g in range(G):
        rs, rr = slice(512 * g, 512 * (g + 1)), slice(256 * g, 256 * (g + 1))
        wqall = np.concatenate([Wq_up[rs].T, Wq_rope[rr].T], axis=1)
        per_g.append({
            "wqd8": pmaj(wqdT[:, QSH * g : QSH * (g + 1)] * 64.0, 16).astype(F8),
            "wkvd": pmaj(wkvdT[:, KSH * g : KSH * (g + 1)], 16).astype(BF16),
            "wqf": pmaj(wqall * (s2 * 64.0), 12).astype(F8),
            "wku": pmaj(Wk_up[rs].T * s2, 4).astype(BF16),
            "wvu": pmaj(Wv_up[rs].T, 4).astype(BF16),
            "wo": pmaj(Wo[:, rs].T, 4).astype(BF16),
        })
    # x as [p, panel, kt, s] partition-major flat
    def xprep(a):
        # a: [D, S] -> [128, NP*16*PAN]
        return np.ascontiguousarray(
            a.reshape(16, P, 4, PAN).transpose(1, 2, 0, 3).reshape(P, -1))

    xTs = [x[b].T for b in range(B)]
    xps = [xprep(a).astype(BF16) for a in xTs]
    xp8s = [xprep(a).astype(F8) for a in xTs]

    in_maps = []
    for c in range(8):
        b, g = c // G, c % G
        m = dict(shared)
        m.update(per_g[g])
        m["xp"] = xps[b]
        m["xp8"] = xp8s[b]
        in_maps.append(m)
    return in_maps


def kernel(**inputs):
    from concourse.bass_utils import run_bass_kernel_spmd

    if "nc" not in _cache:
        _cache["nc"] = _build_module()
    nc = _cache["nc"]

    in_maps = _prep_inputs(**inputs)
    res = None
    for attempt in range(3):
        try:
            res = run_bass_kernel_spmd(nc, in_maps, core_ids=list(range(8)))
            break
        except Exception:
            if attempt == 2:
                raise
    out = np.zeros((B, S, D), np.float32)
    for c in range(8):
        out[c // G] += res.results[c]["y"].astype(np.float32)
    return out
